# revision 1
# baseline (speedup 1.0000x reference)
"""HNetLoss on 8 Trainium2 NeuronCores.

Structure of the computation (see reference): the homography maps pixel
(x, y) -> (xp, yp) with denominator den = p5*y + 1 and yp = (p3*y+p4)/den —
both depend ONLY on the row y.  Within a row, xp = alpha*x + beta is affine
in the column index x.  Hence the per-(batch, lane) polynomial fits and
losses are fully determined by three per-(batch, row, lane) reductions over
the columns:

    c  = sum_x [label == lane]
    S1 = sum_x (x-256) * [label == lane]
    S2 = sum_x (x-256)^2 * [label == lane]

The device kernel computes exactly those masked reductions (the only part
that touches the 8 MiB label field); the remaining math is O(B*H*L) scalar
work done on host in float64.

Device strategy (pure data parallelism, batch b -> core b):
  - labels are cast to bf16 (values 0..5, exact) and transposed on host so
    the column index x lies on SBUF partitions: tile [128, 4*256].
  - VectorE builds the 5 lane masks with tensor_scalar is_equal (bf16, 4x).
  - TensorE contracts masks against a tiny stationary [128, 4] of column
    weights [1, x-256, hi((x-256)^2), lo((x-256)^2)] (hi/lo split keeps the
    squared weights exact in bf16), accumulating the four x-chunks in PSUM.
  - per-lane PSUM [4, 256] is copied to SBUF and DMA'd out as [4, 1280] f32.
"""

import sys

import numpy as np

try:
    import concourse.bass as bass  # noqa: F401
except ModuleNotFoundError:  # pragma: no cover
    sys.path.insert(0, "/opt/trn_rl_repo")

import ml_dtypes

import concourse.bacc as bacc
import concourse.bass as bass
import concourse.mybir as mybir
import concourse.tile as tile

ORDER = 3
N_LANES = 5
EPS_DEN = 1e-5
RIDGE = 1e-6

B, H, W = 8, 256, 512
N_CORES = 8
XC = 256.0  # centering offset for the column weights (keeps bf16 exact)
N_CHUNKS = W // 128

BF16 = mybir.dt.bfloat16
F32 = mybir.dt.float32


LABW = N_CHUNKS * H  # label columns in the packed input tile
INW = LABW + N_CHUNKS * 4  # + 16 weight columns


def _build_program(copy_engine: str = "scalar") -> bass.Bass:
    # Bacc (not raw Bass): its compile() splits multi-wait sync lists into
    # event-semaphore chains — TRN2 allows only 1 wait per instruction, and
    # the Tile kernel-tail Drain alone needs one wait per engine/DMA used.
    nc = bacc.Bacc("TRN2", target_bir_lowering=False)
    inp_d = nc.declare_dram_parameter("inp", [128, INW], BF16, isOutput=False)
    out_d = nc.declare_dram_parameter("moments", [128, 2 * H], F32, isOutput=True)

    with tile.TileContext(nc) as tc:
        with (
            tc.tile_pool(name="io", bufs=1) as io_pool,
            tc.tile_pool(name="masks", bufs=N_LANES) as mask_pool,
            tc.tile_pool(name="psum", bufs=2, space="PSUM") as psum_pool,
        ):
            inb = io_pool.tile([128, INW], BF16, tag="inb")
            outb = io_pool.tile([128, 2 * H], F32, tag="outb")
            nc.sync.dma_start(inb[:], inp_d[:])
            lab = inb[:, :LABW]
            wxt = inb[:, LABW:]
            masks = []
            for lane in range(N_LANES):
                mask = mask_pool.tile([128, LABW], BF16)
                nc.vector.tensor_scalar(
                    mask[:], lab, float(lane + 1), None, mybir.AluOpType.is_equal
                )
                masks.append(mask)
            # lanes 0-3 run CONCURRENTLY in the four 32-col PE groups
            # (tile_position col tiling); lane j accumulates its 4 x-chunks
            # into psA partitions [32j, 32j+4).  c-outer issue order keeps
            # all four column groups streaming simultaneously.
            psA = psum_pool.tile([128, H], F32, tag="psA")
            psB = psum_pool.tile([128, H], F32, tag="psB")
            # initialize the never-written rows between the 4-row groups so
            # the full-tile copies read defined memory; runs on DVE while the
            # input DMA is still in flight
            nc.vector.memset(psA[:], 0.0)
            nc.vector.memset(psB[:], 0.0)
            for c in range(N_CHUNKS):
                for j in range(4):
                    nc.tensor.matmul(
                        psA[32 * j : 32 * j + 4, :],
                        wxt[:, 4 * c : 4 * c + 4],
                        masks[j][:, H * c : H * c + H],
                        start=(c == 0),
                        stop=(c == N_CHUNKS - 1),
                        tile_position=(0, 32 * j),
                        # lanes touch disjoint partitions; the group check is
                        # bank-granular but has_written is per-element
                        skip_group_check=True,
                    )
            # lane 4: each x-chunk is an independent single-matmul partial in
            # its own column group (all 4 concurrent); host sums the chunks
            for c in range(N_CHUNKS):
                nc.tensor.matmul(
                    psB[32 * c : 32 * c + 4, :],
                    wxt[:, 4 * c : 4 * c + 4],
                    masks[4][:, H * c : H * c + H],
                    start=True,
                    stop=True,
                    tile_position=(0, 32 * c),
                    skip_group_check=True,
                )
            if copy_engine == "scalar":
                nc.scalar.copy(outb[:, :H], psA[:])
                nc.scalar.copy(outb[:, H:], psB[:])
            else:
                nc.vector.tensor_copy(outb[:, :H], psA[:])
                nc.vector.tensor_copy(outb[:, H:], psB[:])
            nc.sync.dma_start(out_d[:], outb[:])
    nc.compile()
    return nc


def _host_prep(instance_label: np.ndarray):
    """Build per-core input maps: transposed bf16 labels + column weights."""
    lab = np.asarray(instance_label)
    # weights, shared by all cores: wx[p, 4c+j] = w_j(x=128c+p)
    x = np.arange(W, dtype=np.float64)
    xc = x - XC
    xc2 = xc * xc
    hi = xc2.astype(ml_dtypes.bfloat16)
    lo = (xc2 - hi.astype(np.float64)).astype(ml_dtypes.bfloat16)
    wx = np.empty((W, 4), dtype=ml_dtypes.bfloat16)
    wx[:, 0] = 1.0
    wx[:, 1] = xc.astype(ml_dtypes.bfloat16)
    wx[:, 2] = hi
    wx[:, 3] = lo
    wx = wx.reshape(N_CHUNKS, 128, 4).transpose(1, 0, 2).reshape(128, N_CHUNKS * 4)

    in_maps = []
    for b in range(B):
        lt = lab[b].T.astype(ml_dtypes.bfloat16)  # [W, H], values 0..5 exact
        lt = lt.reshape(N_CHUNKS, 128, H).transpose(1, 0, 2).reshape(128, N_CHUNKS * H)
        packed = np.concatenate([lt, wx], axis=1)  # [128, INW]
        in_maps.append({"inp": np.ascontiguousarray(packed)})
    return in_maps


def _decode_moments(raw: np.ndarray) -> np.ndarray:
    """Device output [128, 2H] -> canonical [4, N_LANES*H] (f64).

    row 32g+i, cols 0:H  = lane g (col groups 0-3), moment-row i
    row 32g+i, cols H:2H = lane 4's x-chunk g partial, moment-row i
    """
    raw = raw.astype(np.float64).reshape(4, 32, 2 * H)[:, :4, :]  # [g, i, 2H]
    out = np.empty((4, N_LANES * H), np.float64)
    for lane in range(4):
        out[:, H * lane : H * lane + H] = raw[lane, :, :H]
    out[:, H * 4 :] = raw[:, :, H:].sum(axis=0)
    return out


def _finalize(hnet_params: np.ndarray, moments: np.ndarray) -> np.float32:
    """Host-side final math in float64.

    moments: [B, 4, N_LANES*H] f32 device output;
             row j, col H*l+r = sum_x w_j(x) * [label[b,r,x] == l+1]
    """
    p = np.asarray(hnet_params, dtype=np.float64)
    m = moments.astype(np.float64).reshape(B, 4, N_LANES, H).transpose(0, 2, 1, 3)
    c = m[:, :, 0, :]  # [B,L,H]
    S1c = m[:, :, 1, :]
    S2c = m[:, :, 2, :] + m[:, :, 3, :]
    S1 = S1c + XC * c
    S2 = S2c + 2.0 * XC * S1c + XC * XC * c

    r = np.arange(H, dtype=np.float64)
    # match the reference's f32 denominator computation + clamp
    p32 = np.asarray(hnet_params, dtype=np.float32)
    den32 = (p32[:, 5:6] * r.astype(np.float32)[None, :]) + np.float32(1.0)
    den = np.where(np.abs(den32) < EPS_DEN, np.float32(EPS_DEN), den32).astype(
        np.float64
    )
    alpha = p[:, 0:1] / den  # [B,H]
    beta = (p[:, 1:2] * r[None, :] + p[:, 2:3]) / den
    yp = (p[:, 3:4] * r[None, :] + p[:, 4:5]) / den

    al = alpha[:, None, :]  # [B,1,H]
    be = beta[:, None, :]
    Sx = al * S1 + be * c
    Sxx = al * al * S2 + 2 * al * be * S1 + be * be * c

    ypb = yp[:, None, :]  # [B,1,H]
    cnt = c.sum(-1)  # [B,L]
    s = np.stack([(c * ypb**k).sum(-1) for k in range(7)], axis=-1)  # [B,L,7]
    t = np.stack([(Sx * ypb**q).sum(-1) for q in range(4)], axis=-1)  # [B,L,4]
    v = (c * np.abs(den)[:, None, :]).sum(-1)  # [B,L]

    k = ORDER + 1
    A0 = np.empty((B, N_LANES, k, k))
    for i in range(k):
        for j in range(k):
            A0[:, :, i, j] = s[:, :, 6 - i - j]
    rhs = np.stack([t[:, :, 3 - i] for i in range(k)], axis=-1)  # [B,L,4]
    A = A0 + RIDGE * np.eye(k)
    w = np.linalg.solve(A, rhs[..., None])[..., 0]  # [B,L,4]

    xpred = sum(w[:, :, i, None] * ypb ** (3 - i) for i in range(k))  # [B,L,H]
    rss = (Sxx - 2 * xpred * Sx + xpred * xpred * c).sum(-1)  # [B,L]

    cnt_safe = np.maximum(cnt, 1.0)
    lane_loss = (rss / cnt_safe) * (v / cnt_safe)
    valid = (cnt >= ORDER + 1).astype(np.float64)
    nv = valid.sum()
    loss = (valid * lane_loss).sum() / max(nv, 1.0) if nv > 0 else 0.0
    return np.float32(loss)


def _run_device(in_maps, trace: bool = False, trace_cores=None):
    from concourse import bass_utils

    nc = _build_program()
    res = bass_utils.run_bass_kernel_spmd(
        nc,
        in_maps,
        core_ids=list(range(N_CORES)),
        trace=trace,
        trace_cores=trace_cores,
    )
    return res


def kernel(hnet_params: np.ndarray, instance_label: np.ndarray) -> np.ndarray:
    in_maps = _host_prep(instance_label)
    res = _run_device(in_maps)
    moments = np.stack(
        [_decode_moments(np.asarray(res.results[b]["moments"])) for b in range(B)]
    )
    return _finalize(hnet_params, moments)


if __name__ == "__main__":
    # quick CoreSim correctness check against a numpy golden model
    from concourse.bass_interp import CoreSim

    rng = np.random.default_rng(0)
    lab_full = rng.integers(0, 6, size=(B, H, W)).astype(np.int64)
    in_maps = _host_prep(lab_full)

    nc = _build_program()
    sim = CoreSim(nc)
    sim.tensor("inp")[:] = in_maps[0]["inp"]
    sim.simulate()
    mom = _decode_moments(np.asarray(sim.tensor("moments")))

    # golden for batch 0
    x = np.arange(W, dtype=np.float64)
    xc = x - XC
    golden = np.zeros((4, N_LANES * H))
    hi = (xc * xc).astype(ml_dtypes.bfloat16).astype(np.float64)
    lo = (xc * xc) - hi
    for lane in range(N_LANES):
        msk = lab_full[0] == (lane + 1)  # [H, W]
        golden[0, H * lane : H * (lane + 1)] = msk.sum(1)
        golden[1, H * lane : H * (lane + 1)] = (msk * xc).sum(1)
        golden[2, H * lane : H * (lane + 1)] = (msk * hi).sum(1)
        golden[3, H * lane : H * (lane + 1)] = (msk * lo).sum(1)
    err = np.abs(mom - golden)
    rel = err.max() / max(np.abs(golden).max(), 1)
    print("max abs err:", err.max(), "max rel:", rel)
    assert rel < 1e-6, "CoreSim mismatch"
    print("CoreSim moments check PASSED")



# revision 3
# speedup vs baseline: 1.7410x; 1.7410x over previous
"""HNetLoss on 8 Trainium2 NeuronCores — v3 (fp8 DoubleRow + byte planes).

Math: per (batch, lane, row) the loss reduces to masked column moments
S_j[l, r] = sum_x w_j(x) [lab[r,x]==l] for w in {1, xc, xc^2} (xc=x-256);
the rest is exact host math (see _finalize).

Device scheme — five independent label functions, all vanishing at 0:
  * The bf16 label tile BITCAST to fp8 yields two FREE functions: the
    byte planes decode as f_lo(lab) = [0,-0,0,2,-0,-0.125] and
    f_hi(lab) = [0,1.875,2,2,2,2].
  * Three DVE is_equal masks (lanes 4,1,2); a bf16 1.0 mask's payload
    byte is fp8 1.875.
  Host solves the well-conditioned 5x5 system for per-lane moments.

PE: all contractions are fp8 MatmulPerfMode.DoubleRow (0.5 cyc/row).
TRN2 dual-fp8 ISA restrictions (reverse-engineered from neuronxcc):
exactly 16 stationary columns, k-tile weight stride 16, PSUM dst
partition 0 — so every DR matmul lands in PSUM rows 0:16.  Two moment
sets share each 256-col PSUM region via complementary zero-padded
stationary columns (set A rows 0:7, set B rows 8:15); the zero columns
write zeros, so no PSUM memsets are needed anywhere.

Input is split into two DMAs (chunk pair 0-1 + weights, then 2-3) so
mask/matmul work on the first half overlaps the second transfer.
Outputs: out1 [16, 512] = [lo|hi] rows 0:7, [m1|m2] rows 8:15;
out2 [7, 256] = m4.
"""

import sys

import numpy as np

try:
    import concourse.bass as bass  # noqa: F401
except ModuleNotFoundError:  # pragma: no cover
    sys.path.insert(0, "/opt/trn_rl_repo")

import ml_dtypes

import concourse.bacc as bacc
import concourse.bass as bass
import concourse.mybir as mybir
import concourse.tile as tile

ORDER = 3
N_LANES = 5
EPS_DEN = 1e-5
RIDGE = 1e-6

B, H, W = 8, 256, 512
N_CORES = 8
XC = 256.0
N_CHUNKS = W // 128

BF16 = mybir.dt.bfloat16
FP8 = mybir.dt.float8e4
F32 = mybir.dt.float32

LABW = N_CHUNKS * H  # 1024 label columns
NW8 = 7  # real fp8 weight columns: [1, xh, xl, ah, al, bh, bl]
MCOL = 16  # dual-fp8 ldweights requires exactly 16 stationary columns
ROWB = 8  # row offset of the second moment set within a PSUM region
WCOLS = 2 * N_CHUNKS * MCOL  # fp8 weight cols (A and B variants)
HALFW = LABW // 2  # label cols per input half (chunk pair)
INWA = HALFW + WCOLS // 2  # first half also carries the fp8 weights

# fp8 byte-plane decodes of bf16 label values 0..5 (lo byte, hi byte)
F_LO = np.array([0.0, 0.0, 0.0, 2.0, 0.0, -0.125])
F_HI = np.array([0.0, 1.875, 2.0, 2.0, 2.0, 2.0])
MSCALE = 1.875  # payload byte of bf16 1.0 (0x3F80 -> 0x3F = 1.875)


def _build_program() -> bass.Bass:
    nc = bacc.Bacc("TRN2", target_bir_lowering=False)
    inpa_d = nc.declare_dram_parameter("inpa", [128, INWA], BF16, isOutput=False)
    inpb_d = nc.declare_dram_parameter("inpb", [128, HALFW], BF16, isOutput=False)
    out1_d = nc.declare_dram_parameter("out1", [16, 512], F32, isOutput=True)
    out2_d = nc.declare_dram_parameter("out2", [7, 256], F32, isOutput=True)

    with tile.TileContext(nc) as tc:
        with (
            tc.tile_pool(name="io", bufs=1) as io_pool,
            tc.tile_pool(name="masks", bufs=3) as mask_pool,
            tc.tile_pool(name="psum", bufs=2, space="PSUM") as psum_pool,
        ):
            inba = io_pool.tile([128, INWA], BF16, tag="inba")
            inbb = io_pool.tile([128, HALFW], BF16, tag="inbb")
            outba = io_pool.tile([128, 512], F32, tag="outba")
            outbb = io_pool.tile([128, 256], F32, tag="outbb")
            bank1 = psum_pool.tile([128, 512], F32, tag="bank1")
            bank2 = psum_pool.tile([128, 256], F32, tag="bank2")

            # Each bank runs ONE accumulation group (CoreSim's pending-zero
            # tracking is 2KB-row granular): only the first matmul sets
            # start, only the last sets stop, and the early memsets below
            # give the hardware zeros to accumulate onto for regions the
            # start-matmul does not touch.
            nc.vector.memset(bank1[:], 0.0)
            nc.vector.memset(bank2[:], 0.0)

            # split input: chunks 0-1 (+ weights) land ~600ns before 2-3,
            # so masks/matmuls on the first half overlap the second transfer
            nc.sync.dma_start(inba[:], inpa_d[:])
            nc.sync.dma_start(inbb[:], inpb_d[:])

            labh = [inba[:, :HALFW], inbb[:]]  # chunk pairs (0,1), (2,3)
            # fp8 weights: per chunk MCOL cols variant A (w in cols 0:7)
            # then MCOL cols variant B (w in cols 8:15); [128, 2, 16] pairs
            wf8 = inba[:, HALFW:].bitcast(FP8)  # [128, 128]
            w4 = wf8.rearrange("p (c v j) -> p c v j", c=N_CHUNKS, v=2, j=MCOL)
            wA = [w4[:, 0:2, 0, :], w4[:, 2:4, 0, :]]
            wB = [w4[:, 0:2, 1, :], w4[:, 2:4, 1, :]]

            # DVE mask order {4, 1, 2} per half: lane 4 first so its bank2
            # matmuls + tail copy + small DMA finish early, off the chain
            masks = {}
            for lane in (4, 1, 2):
                m = mask_pool.tile([128, LABW], BF16, tag=f"m{lane}")
                masks[lane] = m
            for half in range(2):
                for lane in (4, 1, 2):
                    nc.vector.tensor_scalar(
                        masks[lane][:, HALFW * half : HALFW * (half + 1)],
                        labh[half],
                        float(lane),
                        None,
                        mybir.AluOpType.is_equal,
                    )

            DR = mybir.MatmulPerfMode.DoubleRow

            def plane(src_ap, s):
                v = src_ap.bitcast(FP8).rearrange(
                    "p (c r s) -> p c r s", c=2, r=H, s=2
                )
                return v[:, :, :, s]

            def mhalf(lane, h):
                return plane(masks[lane][:, HALFW * h : HALFW * (h + 1)], 1)

            # jobs: (psum region, weight variant, moving-for-half, role):
            # role 'bank1-first' starts bank1's group, 'bank1-last' stops it;
            # bank2 has its own group (m4 only)
            jobs = [
                (bank1[0:MCOL, 0:256], wA, lambda h: plane(labh[h], 0), "b1s"),
                (bank1[0:MCOL, 256:512], wA, lambda h: plane(labh[h], 1), ""),
                (bank2[0:MCOL, 0:256], wA, lambda h: mhalf(4, h), "b2"),
                (bank1[0:MCOL, 0:256], wB, lambda h: mhalf(1, h), ""),
                (bank1[0:MCOL, 256:512], wB, lambda h: mhalf(2, h), "b1e"),
            ]
            for half in range(2):
                for out_ap, w, moving, role in jobs:
                    start = (role in ("b1s", "b2")) and half == 0
                    stop = (role in ("b1e", "b2")) and half == 1
                    nc.tensor.matmul(
                        out_ap,
                        w[half],
                        moving(half),
                        start=start,
                        stop=stop,
                        perf_mode=DR,
                        tile_position=(0, 0),
                        skip_group_check=True,
                    )

            # PSUM -> SBUF staging on separate tiles (no WAW between them);
            # the small m4 piece completes early and its DMA rides first
            nc.vector.tensor_copy(outbb[0:NW8, :], bank2[0:NW8, 0:256])
            nc.scalar.copy(outba[0:MCOL, :], bank1[0:MCOL, :])

            nc.sync.dma_start(out2_d[:], outbb[0:NW8, :])
            nc.sync.dma_start(out1_d[:], outba[0:MCOL, :])

    # Drop the framework's const-tile memsets from the entry block: nothing
    # in this program reads the const APs (all scalar operands are literal
    # immediates), and the 4 serial Pool memsets (95ns each) gate the entry
    # barrier and hence the input DMA issue.
    blk0 = nc.m.functions[0].blocks[0]
    dead = [
        i
        for i, inst in enumerate(blk0.instructions)
        if inst.opcode == "Memset"
        and any("const-" in str(o) for o in inst.outs)
    ]
    if dead:
        used = set()
        for fn in nc.m.functions:
            for blk in fn.blocks:
                for inst in blk.instructions:
                    if inst.opcode == "Memset":
                        continue
                    for ap in list(inst.ins) + list(inst.outs):
                        used.add(str(ap))
        assert not any("const-" in u for u in used), "const APs are used"
        for i in reversed(dead):
            del blk0.instructions[i]
    # With the memsets gone the entry all-engine barrier synchronizes idle
    # engines only; drop it too so the input DMA issues immediately.
    bar = [
        i
        for i, inst in enumerate(blk0.instructions)
        if inst.opcode in ("Drain", "EventSemaphore")
    ]
    for i in reversed(bar):
        del blk0.instructions[i]
    # Exit block: keep the SP queue-drain waits (output-DMA completion) but
    # drop the two all-engine barrier ping-pong rounds and the semaphore
    # range clear — they only matter for re-executing a still-loaded NEFF.
    blk2 = nc.m.functions[0].blocks[2]
    tail = [
        i
        for i, inst in enumerate(blk2.instructions)
        if "barrier" in inst.concise()
        or inst.opcode == "EVENT_SEMAPHORE_RANGE_CLEAR"
        or (inst.opcode == "Drain" and "is_reset_sema=True" in inst.concise())
    ]
    for i in reversed(tail):
        del blk2.instructions[i]
    nc.compile()
    return nc


def _fp8_hi_lo(vals: np.ndarray, clip: float = 240.0):
    """Exact v = hi + lo split with both parts fp8 e4m3 (max normal 240)."""
    e4 = ml_dtypes.float8_e4m3
    hi = np.clip(vals, -clip, clip).astype(e4)
    hi64 = hi.astype(np.float64)
    lo = (vals - hi64).astype(e4)
    assert np.all(lo.astype(np.float64) + hi64 == vals), "fp8 split not exact"
    return hi, lo


def _fp8_weights() -> np.ndarray:
    """[128, WCOLS] fp8: per chunk, 16 cols variant A ([1,xh,xl,ah,al,bh,bl]
    in cols 0:7) then 16 cols variant B (same weights in cols 8:15)."""
    e4 = ml_dtypes.float8_e4m3
    x = np.arange(W, dtype=np.float64)
    xc = x - XC
    xc2 = xc * xc
    a = np.floor(xc2 / 256.0)
    b = xc2 - 256.0 * a
    xh, xl = _fp8_hi_lo(xc)
    ah, al = _fp8_hi_lo(a)
    bh, bl = _fp8_hi_lo(b)
    wreal = np.zeros((W, NW8), dtype=e4)
    wreal[:, 0] = 1.0
    wreal[:, 1] = xh
    wreal[:, 2] = xl
    wreal[:, 3] = ah
    wreal[:, 4] = al
    wreal[:, 5] = bh
    wreal[:, 6] = bl
    w = np.zeros((W, 2, MCOL), dtype=e4)
    w[:, 0, :NW8] = wreal
    w[:, 1, ROWB : ROWB + NW8] = wreal
    # [x, v, j] -> [p, c, v, j] -> [128, WCOLS]
    return np.ascontiguousarray(
        w.reshape(N_CHUNKS, 128, 2 * MCOL).transpose(1, 0, 2).reshape(128, WCOLS)
    )


def _host_prep(instance_label: np.ndarray):
    lab = np.asarray(instance_label)
    wf8 = _fp8_weights()
    in_maps = []
    for b in range(B):
        lt = lab[b].T.astype(ml_dtypes.bfloat16)  # [W, H]
        lt = lt.reshape(N_CHUNKS, 128, H).transpose(1, 0, 2).reshape(128, LABW)
        inpa = np.empty((128, INWA), dtype=ml_dtypes.bfloat16)
        inpa[:, :HALFW] = lt[:, :HALFW]
        inpa[:, HALFW:].view(np.uint8)[:] = wf8.view(np.uint8)
        in_maps.append({"inpa": inpa, "inpb": np.ascontiguousarray(lt[:, HALFW:])})
    return in_maps


def _decode_moments(raw1: np.ndarray, raw2: np.ndarray) -> np.ndarray:
    """Device outputs -> per-lane moments [3, N_LANES, H] f64.

    raw1 [16, 512]: rows 0:7 = [lo | hi], rows 8:15 = [m1 | m2];
    raw2 [7, 256]: m4.  Moment index: 0 = count, 1 = sum xc, 2 = sum xc^2.
    """
    g1 = raw1.astype(np.float64)
    g2 = raw2.astype(np.float64)

    def comb(t):  # [7, H] fp8-moment rows -> [3, H]
        return np.stack(
            [t[0], t[1] + t[2], 256.0 * (t[3] + t[4]) + t[5] + t[6]]
        )

    t_lo = comb(g1[0:NW8, 0:256])
    t_hi = comb(g1[0:NW8, 256:512])
    t_m1 = comb(g1[ROWB : ROWB + NW8, 0:256]) / MSCALE
    t_m2 = comb(g1[ROWB : ROWB + NW8, 256:512]) / MSCALE
    t_m4 = comb(g2) / MSCALE

    # unmix: u_l = per-lane moments; lanes 1,2,4 direct, 3 & 5 from planes:
    #   f_lo: 2*u3 - 0.125*u5            = t_lo
    #   f_hi: 1.875*u1 + 2*(u2+u3+u4+u5) = t_hi
    u1, u2, u4 = t_m1, t_m2, t_m4
    a = t_hi - F_HI[1] * u1 - F_HI[2] * u2 - F_HI[4] * u4  # 2*u3 + 2*u5
    u5 = (a - t_lo) / 2.125
    u3 = (t_lo + 0.125 * u5) / 2.0
    return np.stack([u1, u2, u3, u4, u5], axis=1)  # [3, L, H]


def _finalize(hnet_params: np.ndarray, moments: np.ndarray) -> np.float32:
    """moments: [B, 3, L, H] f64 (count, S1 about XC, S2 about XC)."""
    p = np.asarray(hnet_params, dtype=np.float64)
    c = moments[:, 0]  # [B, L, H]
    S1c = moments[:, 1]
    S2c = moments[:, 2]
    S1 = S1c + XC * c
    S2 = S2c + 2.0 * XC * S1c + XC * XC * c

    r = np.arange(H, dtype=np.float64)
    p32 = np.asarray(hnet_params, dtype=np.float32)
    den32 = (p32[:, 5:6] * r.astype(np.float32)[None, :]) + np.float32(1.0)
    den = np.where(np.abs(den32) < EPS_DEN, np.float32(EPS_DEN), den32).astype(
        np.float64
    )
    alpha = p[:, 0:1] / den  # [B,H]
    beta = (p[:, 1:2] * r[None, :] + p[:, 2:3]) / den
    yp = (p[:, 3:4] * r[None, :] + p[:, 4:5]) / den

    al = alpha[:, None, :]
    be = beta[:, None, :]
    Sx = al * S1 + be * c
    Sxx = al * al * S2 + 2 * al * be * S1 + be * be * c

    ypb = yp[:, None, :]
    cnt = c.sum(-1)  # [B,L]
    s = np.stack([(c * ypb**k).sum(-1) for k in range(7)], axis=-1)
    t = np.stack([(Sx * ypb**q).sum(-1) for q in range(4)], axis=-1)
    v = (c * np.abs(den)[:, None, :]).sum(-1)

    k = ORDER + 1
    A0 = np.empty((B, N_LANES, k, k))
    for i in range(k):
        for j in range(k):
            A0[:, :, i, j] = s[:, :, 6 - i - j]
    rhs = np.stack([t[:, :, 3 - i] for i in range(k)], axis=-1)
    A = A0 + RIDGE * np.eye(k)
    w = np.linalg.solve(A, rhs[..., None])[..., 0]

    xpred = sum(w[:, :, i, None] * ypb ** (3 - i) for i in range(k))
    rss = (Sxx - 2 * xpred * Sx + xpred * xpred * c).sum(-1)

    cnt_safe = np.maximum(cnt, 1.0)
    lane_loss = (rss / cnt_safe) * (v / cnt_safe)
    valid = (cnt >= ORDER + 1).astype(np.float64)
    nv = valid.sum()
    loss = (valid * lane_loss).sum() / max(nv, 1.0) if nv > 0 else 0.0
    return np.float32(loss)


def _run_device(in_maps, trace: bool = False, trace_cores=None):
    from concourse import bass_utils

    nc = _build_program()
    res = bass_utils.run_bass_kernel_spmd(
        nc,
        in_maps,
        core_ids=list(range(N_CORES)),
        trace=trace,
        trace_cores=trace_cores,
    )
    return res


def kernel(hnet_params: np.ndarray, instance_label: np.ndarray) -> np.ndarray:
    in_maps = _host_prep(instance_label)
    res = _run_device(in_maps)
    moments = np.stack(
        [
            _decode_moments(
                np.asarray(res.results[b]["out1"]),
                np.asarray(res.results[b]["out2"]),
            )
            for b in range(B)
        ]
    )
    return _finalize(hnet_params, moments)


def _golden_moments(lab_b: np.ndarray) -> np.ndarray:
    """Numpy golden for one batch: [3, L, H] exact moments."""
    x = np.arange(W, dtype=np.float64)
    xc = x - XC
    out = np.zeros((3, N_LANES, H))
    for lane in range(N_LANES):
        msk = lab_b == (lane + 1)  # [H, W]
        out[0, lane] = msk.sum(1)
        out[1, lane] = (msk * xc).sum(1)
        out[2, lane] = (msk * xc * xc).sum(1)
    return out


if __name__ == "__main__":
    from concourse.bass_interp import CoreSim

    rng = np.random.default_rng(0)
    lab_full = rng.integers(0, 6, size=(B, H, W)).astype(np.int64)
    in_maps = _host_prep(lab_full)

    nc = _build_program()
    sim = CoreSim(nc)
    sim.tensor("inpa")[:] = in_maps[0]["inpa"]
    sim.tensor("inpb")[:] = in_maps[0]["inpb"]
    sim.simulate()
    mom = _decode_moments(
        np.asarray(sim.tensor("out1")), np.asarray(sim.tensor("out2"))
    )

    golden = _golden_moments(lab_full[0])
    err = np.abs(mom - golden)
    rel = err.max() / max(np.abs(golden).max(), 1)
    print("max abs err:", err.max(), "max rel:", rel)
    assert rel < 1e-6, "CoreSim moments mismatch"
    print("CoreSim moments check PASSED")


# revision 4
# speedup vs baseline: 1.7989x; 1.0333x over previous
"""HNetLoss on 8 Trainium2 NeuronCores — v3 (fp8 DoubleRow + byte planes).

Math: per (batch, lane, row) the loss reduces to masked column moments
S_j[l, r] = sum_x w_j(x) [lab[r,x]==l] for w in {1, xc, xc^2} (xc=x-256);
the rest is exact host math (see _finalize).

Device scheme — five independent label functions, all vanishing at 0:
  * The bf16 label tile BITCAST to fp8 yields two FREE functions: the
    byte planes decode as f_lo(lab) = [0,-0,0,2,-0,-0.125] and
    f_hi(lab) = [0,1.875,2,2,2,2].
  * Three DVE is_equal masks (lanes 4,1,2); a bf16 1.0 mask's payload
    byte is fp8 1.875.
  Host solves the well-conditioned 5x5 system for per-lane moments.

PE: all contractions are fp8 MatmulPerfMode.DoubleRow (0.5 cyc/row).
TRN2 dual-fp8 ISA restrictions (reverse-engineered from neuronxcc):
exactly 16 stationary columns, k-tile weight stride 16, PSUM dst
partition 0 — so every DR matmul lands in PSUM rows 0:16.  Two moment
sets share each 256-col PSUM region via complementary zero-padded
stationary columns (set A rows 0:7, set B rows 8:15); the zero columns
write zeros, so no PSUM memsets are needed anywhere.

Input is split into two DMAs (chunk pair 0-1 + weights, then 2-3) so
mask/matmul work on the first half overlaps the second transfer.
Outputs: out1 [16, 512] = [lo|hi] rows 0:7, [m1|m2] rows 8:15;
out2 [7, 256] = m4.
"""

import sys

import numpy as np

try:
    import concourse.bass as bass  # noqa: F401
except ModuleNotFoundError:  # pragma: no cover
    sys.path.insert(0, "/opt/trn_rl_repo")

import ml_dtypes

import concourse.bacc as bacc
import concourse.bass as bass
import concourse.mybir as mybir
import concourse.tile as tile

ORDER = 3
N_LANES = 5
EPS_DEN = 1e-5
RIDGE = 1e-6

B, H, W = 8, 256, 512
N_CORES = 8
XC = 256.0
N_CHUNKS = W // 128

BF16 = mybir.dt.bfloat16
FP8 = mybir.dt.float8e4
F32 = mybir.dt.float32

LABW = N_CHUNKS * H  # 1024 label columns
NW8 = 7  # real fp8 weight columns: [1, xh, xl, ah, al, bh, bl]
MCOL = 16  # dual-fp8 ldweights requires exactly 16 stationary columns
ROWB = 8  # row offset of the second moment set within a PSUM region
WCOLS = 2 * N_CHUNKS * MCOL  # fp8 weight cols (A and B variants)
HALFW = LABW // 2  # label cols per input half (chunk pair)
INWA = HALFW + WCOLS // 2  # first half also carries the fp8 weights

# fp8 byte-plane decodes of bf16 label values 0..5 (lo byte, hi byte)
F_LO = np.array([0.0, 0.0, 0.0, 2.0, 0.0, -0.125])
F_HI = np.array([0.0, 1.875, 2.0, 2.0, 2.0, 2.0])
MSCALE = 1.875  # payload byte of bf16 1.0 (0x3F80 -> 0x3F = 1.875)


def _build_program() -> bass.Bass:
    nc = bacc.Bacc("TRN2", target_bir_lowering=False)
    inpa_d = nc.declare_dram_parameter("inpa", [128, INWA], BF16, isOutput=False)
    inpb_d = nc.declare_dram_parameter("inpb", [128, HALFW], BF16, isOutput=False)
    out1_d = nc.declare_dram_parameter("out1", [16, 512], F32, isOutput=True)
    out2_d = nc.declare_dram_parameter("out2", [7, 256], F32, isOutput=True)

    with tile.TileContext(nc) as tc:
        with (
            tc.tile_pool(name="io", bufs=1) as io_pool,
            tc.tile_pool(name="masks", bufs=3) as mask_pool,
            tc.tile_pool(name="psum", bufs=2, space="PSUM") as psum_pool,
        ):
            inba = io_pool.tile([128, INWA], BF16, tag="inba")
            inbb = io_pool.tile([128, HALFW], BF16, tag="inbb")
            outba = io_pool.tile([128, 512], F32, tag="outba")
            outbb = io_pool.tile([128, 256], F32, tag="outbb")
            bank1 = psum_pool.tile([128, 512], F32, tag="bank1")
            bank2 = psum_pool.tile([128, 256], F32, tag="bank2")

            # Each bank runs ONE accumulation group (CoreSim's pending-zero
            # tracking is 2KB-row granular): only the first matmul sets
            # start, only the last sets stop, and the early memsets below
            # give the hardware zeros to accumulate onto for regions the
            # start-matmul does not touch.
            nc.vector.memset(bank1[:], 0.0)
            nc.vector.memset(bank2[:], 0.0)

            # split input: chunks 0-1 (+ weights) land ~600ns before 2-3,
            # so masks/matmuls on the first half overlap the second transfer.
            # Half B goes through the Pool-engine SWDGE path so its
            # descriptor generation runs in parallel with half A's HWDGE
            # ring instead of serializing behind it.
            nc.sync.dma_start(inba[:], inpa_d[:])
            nc.gpsimd.dma_start(inbb[:], inpb_d[:])

            labh = [inba[:, :HALFW], inbb[:]]  # chunk pairs (0,1), (2,3)
            # fp8 weights: per chunk MCOL cols variant A (w in cols 0:7)
            # then MCOL cols variant B (w in cols 8:15); [128, 2, 16] pairs
            wf8 = inba[:, HALFW:].bitcast(FP8)  # [128, 128]
            w4 = wf8.rearrange("p (c v j) -> p c v j", c=N_CHUNKS, v=2, j=MCOL)
            wA = [w4[:, 0:2, 0, :], w4[:, 2:4, 0, :]]
            wB = [w4[:, 0:2, 1, :], w4[:, 2:4, 1, :]]

            # Masks: DVE runs {m1,m2} x half0, then m4-half1, then {m1,m2} x
            # half1 back-to-back from the moment half0 lands; the otherwise
            # idle Pool engine computes m4-half0 in parallel (slower per
            # element but completely off the DVE critical chain).
            masks = {}
            for lane in (4, 1, 2):
                m = mask_pool.tile([128, LABW], BF16, tag=f"m{lane}")
                masks[lane] = m

            def mask_op(eng, lane, half):
                eng.tensor_scalar(
                    masks[lane][:, HALFW * half : HALFW * (half + 1)],
                    labh[half],
                    float(lane),
                    None,
                    mybir.AluOpType.is_equal,
                )

            mask_op(nc.gpsimd, 4, 0)
            mask_op(nc.vector, 1, 0)
            mask_op(nc.vector, 2, 0)
            mask_op(nc.vector, 4, 1)
            mask_op(nc.vector, 1, 1)
            mask_op(nc.vector, 2, 1)

            DR = mybir.MatmulPerfMode.DoubleRow

            def plane(src_ap, s):
                v = src_ap.bitcast(FP8).rearrange(
                    "p (c r s) -> p c r s", c=2, r=H, s=2
                )
                return v[:, :, :, s]

            def mhalf(lane, h):
                return plane(masks[lane][:, HALFW * h : HALFW * (h + 1)], 1)

            # Matmul schedule: (region, weight variant, moving, half, start,
            # stop).  Bank1's group opens with lo-01 and closes with m2-23;
            # bank2's (m4) opens with the pair-23 half because its pair-01
            # mask comes late from the Pool engine — m4-01 runs between the
            # late masks, keeping both m4 matmuls clear of the tail.
            sched = [
                (bank1[0:MCOL, 0:256], wA, plane(labh[0], 0), 0, True, False),
                (bank1[0:MCOL, 256:512], wA, plane(labh[0], 1), 0, False, False),
                (bank1[0:MCOL, 0:256], wB, mhalf(1, 0), 0, False, False),
                (bank1[0:MCOL, 256:512], wB, mhalf(2, 0), 0, False, False),
                (bank1[0:MCOL, 0:256], wA, plane(labh[1], 0), 1, False, False),
                (bank1[0:MCOL, 256:512], wA, plane(labh[1], 1), 1, False, False),
                (bank2[0:MCOL, 0:256], wA, mhalf(4, 1), 1, True, False),
                (bank1[0:MCOL, 0:256], wB, mhalf(1, 1), 1, False, False),
                (bank2[0:MCOL, 0:256], wA, mhalf(4, 0), 0, False, True),
                (bank1[0:MCOL, 256:512], wB, mhalf(2, 1), 1, False, True),
            ]
            for out_ap, w, moving, half, start, stop in sched:
                nc.tensor.matmul(
                    out_ap,
                    w[half],
                    moving,
                    start=start,
                    stop=stop,
                    perf_mode=DR,
                    tile_position=(0, 0),
                    skip_group_check=True,
                )

            # PSUM -> SBUF staging on separate tiles (no WAW between them).
            # m4 finishes early: stage it on Act and ship via the Pool SWDGE
            # so the critical bank1 copy (DVE) + its HWDGE DMA never wait.
            nc.scalar.copy(outbb[0:NW8, :], bank2[0:NW8, 0:256])
            nc.vector.tensor_copy(outba[0:MCOL, :], bank1[0:MCOL, :])

            nc.gpsimd.dma_start(out2_d[:], outbb[0:NW8, :])
            nc.sync.dma_start(out1_d[:], outba[0:MCOL, :])

    # Drop the framework's const-tile memsets from the entry block: nothing
    # in this program reads the const APs (all scalar operands are literal
    # immediates), and the 4 serial Pool memsets (95ns each) gate the entry
    # barrier and hence the input DMA issue.
    blk0 = nc.m.functions[0].blocks[0]
    dead = [
        i
        for i, inst in enumerate(blk0.instructions)
        if inst.opcode == "Memset"
        and any("const-" in str(o) for o in inst.outs)
    ]
    if dead:
        used = set()
        for fn in nc.m.functions:
            for blk in fn.blocks:
                for inst in blk.instructions:
                    if inst.opcode == "Memset":
                        continue
                    for ap in list(inst.ins) + list(inst.outs):
                        used.add(str(ap))
        assert not any("const-" in u for u in used), "const APs are used"
        for i in reversed(dead):
            del blk0.instructions[i]
    # With the memsets gone the entry all-engine barrier synchronizes idle
    # engines only; drop it too so the input DMA issues immediately.
    bar = [
        i
        for i, inst in enumerate(blk0.instructions)
        if inst.opcode in ("Drain", "EventSemaphore")
    ]
    for i in reversed(bar):
        del blk0.instructions[i]
    # Exit block: keep the SP queue-drain waits (output-DMA completion) but
    # drop the two all-engine barrier ping-pong rounds and the semaphore
    # range clear — they only matter for re-executing a still-loaded NEFF.
    blk2 = nc.m.functions[0].blocks[2]
    tail = [
        i
        for i, inst in enumerate(blk2.instructions)
        if "barrier" in inst.concise()
        or "EVENT_SEMAPHORE_RANGE_CLEAR" in inst.concise()
        or (inst.opcode == "Drain" and "is_reset_sema=True" in inst.concise())
    ]
    for i in reversed(tail):
        del blk2.instructions[i]
    nc.compile()
    return nc


def _fp8_hi_lo(vals: np.ndarray, clip: float = 240.0):
    """Exact v = hi + lo split with both parts fp8 e4m3 (max normal 240)."""
    e4 = ml_dtypes.float8_e4m3
    hi = np.clip(vals, -clip, clip).astype(e4)
    hi64 = hi.astype(np.float64)
    lo = (vals - hi64).astype(e4)
    assert np.all(lo.astype(np.float64) + hi64 == vals), "fp8 split not exact"
    return hi, lo


def _fp8_weights() -> np.ndarray:
    """[128, WCOLS] fp8: per chunk, 16 cols variant A ([1,xh,xl,ah,al,bh,bl]
    in cols 0:7) then 16 cols variant B (same weights in cols 8:15)."""
    e4 = ml_dtypes.float8_e4m3
    x = np.arange(W, dtype=np.float64)
    xc = x - XC
    xc2 = xc * xc
    a = np.floor(xc2 / 256.0)
    b = xc2 - 256.0 * a
    xh, xl = _fp8_hi_lo(xc)
    ah, al = _fp8_hi_lo(a)
    bh, bl = _fp8_hi_lo(b)
    wreal = np.zeros((W, NW8), dtype=e4)
    wreal[:, 0] = 1.0
    wreal[:, 1] = xh
    wreal[:, 2] = xl
    wreal[:, 3] = ah
    wreal[:, 4] = al
    wreal[:, 5] = bh
    wreal[:, 6] = bl
    w = np.zeros((W, 2, MCOL), dtype=e4)
    w[:, 0, :NW8] = wreal
    w[:, 1, ROWB : ROWB + NW8] = wreal
    # [x, v, j] -> [p, c, v, j] -> [128, WCOLS]
    return np.ascontiguousarray(
        w.reshape(N_CHUNKS, 128, 2 * MCOL).transpose(1, 0, 2).reshape(128, WCOLS)
    )


def _host_prep(instance_label: np.ndarray):
    lab = np.asarray(instance_label)
    wf8 = _fp8_weights()
    in_maps = []
    for b in range(B):
        lt = lab[b].T.astype(ml_dtypes.bfloat16)  # [W, H]
        lt = lt.reshape(N_CHUNKS, 128, H).transpose(1, 0, 2).reshape(128, LABW)
        inpa = np.empty((128, INWA), dtype=ml_dtypes.bfloat16)
        inpa[:, :HALFW] = lt[:, :HALFW]
        inpa[:, HALFW:].view(np.uint8)[:] = wf8.view(np.uint8)
        in_maps.append({"inpa": inpa, "inpb": np.ascontiguousarray(lt[:, HALFW:])})
    return in_maps


def _decode_moments(raw1: np.ndarray, raw2: np.ndarray) -> np.ndarray:
    """Device outputs -> per-lane moments [3, N_LANES, H] f64.

    raw1 [16, 512]: rows 0:7 = [lo | hi], rows 8:15 = [m1 | m2];
    raw2 [7, 256]: m4.  Moment index: 0 = count, 1 = sum xc, 2 = sum xc^2.
    """
    g1 = raw1.astype(np.float64)
    g2 = raw2.astype(np.float64)

    def comb(t):  # [7, H] fp8-moment rows -> [3, H]
        return np.stack(
            [t[0], t[1] + t[2], 256.0 * (t[3] + t[4]) + t[5] + t[6]]
        )

    t_lo = comb(g1[0:NW8, 0:256])
    t_hi = comb(g1[0:NW8, 256:512])
    t_m1 = comb(g1[ROWB : ROWB + NW8, 0:256]) / MSCALE
    t_m2 = comb(g1[ROWB : ROWB + NW8, 256:512]) / MSCALE
    t_m4 = comb(g2) / MSCALE

    # unmix: u_l = per-lane moments; lanes 1,2,4 direct, 3 & 5 from planes:
    #   f_lo: 2*u3 - 0.125*u5            = t_lo
    #   f_hi: 1.875*u1 + 2*(u2+u3+u4+u5) = t_hi
    u1, u2, u4 = t_m1, t_m2, t_m4
    a = t_hi - F_HI[1] * u1 - F_HI[2] * u2 - F_HI[4] * u4  # 2*u3 + 2*u5
    u5 = (a - t_lo) / 2.125
    u3 = (t_lo + 0.125 * u5) / 2.0
    return np.stack([u1, u2, u3, u4, u5], axis=1)  # [3, L, H]


def _finalize(hnet_params: np.ndarray, moments: np.ndarray) -> np.float32:
    """moments: [B, 3, L, H] f64 (count, S1 about XC, S2 about XC)."""
    p = np.asarray(hnet_params, dtype=np.float64)
    c = moments[:, 0]  # [B, L, H]
    S1c = moments[:, 1]
    S2c = moments[:, 2]
    S1 = S1c + XC * c
    S2 = S2c + 2.0 * XC * S1c + XC * XC * c

    r = np.arange(H, dtype=np.float64)
    p32 = np.asarray(hnet_params, dtype=np.float32)
    den32 = (p32[:, 5:6] * r.astype(np.float32)[None, :]) + np.float32(1.0)
    den = np.where(np.abs(den32) < EPS_DEN, np.float32(EPS_DEN), den32).astype(
        np.float64
    )
    alpha = p[:, 0:1] / den  # [B,H]
    beta = (p[:, 1:2] * r[None, :] + p[:, 2:3]) / den
    yp = (p[:, 3:4] * r[None, :] + p[:, 4:5]) / den

    al = alpha[:, None, :]
    be = beta[:, None, :]
    Sx = al * S1 + be * c
    Sxx = al * al * S2 + 2 * al * be * S1 + be * be * c

    ypb = yp[:, None, :]
    cnt = c.sum(-1)  # [B,L]
    s = np.stack([(c * ypb**k).sum(-1) for k in range(7)], axis=-1)
    t = np.stack([(Sx * ypb**q).sum(-1) for q in range(4)], axis=-1)
    v = (c * np.abs(den)[:, None, :]).sum(-1)

    k = ORDER + 1
    A0 = np.empty((B, N_LANES, k, k))
    for i in range(k):
        for j in range(k):
            A0[:, :, i, j] = s[:, :, 6 - i - j]
    rhs = np.stack([t[:, :, 3 - i] for i in range(k)], axis=-1)
    A = A0 + RIDGE * np.eye(k)
    w = np.linalg.solve(A, rhs[..., None])[..., 0]

    xpred = sum(w[:, :, i, None] * ypb ** (3 - i) for i in range(k))
    rss = (Sxx - 2 * xpred * Sx + xpred * xpred * c).sum(-1)

    cnt_safe = np.maximum(cnt, 1.0)
    lane_loss = (rss / cnt_safe) * (v / cnt_safe)
    valid = (cnt >= ORDER + 1).astype(np.float64)
    nv = valid.sum()
    loss = (valid * lane_loss).sum() / max(nv, 1.0) if nv > 0 else 0.0
    return np.float32(loss)


def _run_device(in_maps, trace: bool = False, trace_cores=None):
    from concourse import bass_utils

    nc = _build_program()
    res = bass_utils.run_bass_kernel_spmd(
        nc,
        in_maps,
        core_ids=list(range(N_CORES)),
        trace=trace,
        trace_cores=trace_cores,
    )
    return res


def kernel(hnet_params: np.ndarray, instance_label: np.ndarray) -> np.ndarray:
    in_maps = _host_prep(instance_label)
    res = _run_device(in_maps)
    moments = np.stack(
        [
            _decode_moments(
                np.asarray(res.results[b]["out1"]),
                np.asarray(res.results[b]["out2"]),
            )
            for b in range(B)
        ]
    )
    return _finalize(hnet_params, moments)


def _golden_moments(lab_b: np.ndarray) -> np.ndarray:
    """Numpy golden for one batch: [3, L, H] exact moments."""
    x = np.arange(W, dtype=np.float64)
    xc = x - XC
    out = np.zeros((3, N_LANES, H))
    for lane in range(N_LANES):
        msk = lab_b == (lane + 1)  # [H, W]
        out[0, lane] = msk.sum(1)
        out[1, lane] = (msk * xc).sum(1)
        out[2, lane] = (msk * xc * xc).sum(1)
    return out


if __name__ == "__main__":
    from concourse.bass_interp import CoreSim

    rng = np.random.default_rng(0)
    lab_full = rng.integers(0, 6, size=(B, H, W)).astype(np.int64)
    in_maps = _host_prep(lab_full)

    nc = _build_program()
    sim = CoreSim(nc)
    sim.tensor("inpa")[:] = in_maps[0]["inpa"]
    sim.tensor("inpb")[:] = in_maps[0]["inpb"]
    sim.simulate()
    mom = _decode_moments(
        np.asarray(sim.tensor("out1")), np.asarray(sim.tensor("out2"))
    )

    golden = _golden_moments(lab_full[0])
    err = np.abs(mom - golden)
    rel = err.max() / max(np.abs(golden).max(), 1)
    print("max abs err:", err.max(), "max rel:", rel)
    assert rel < 1e-6, "CoreSim moments mismatch"
    print("CoreSim moments check PASSED")


# revision 5
# speedup vs baseline: 1.8063x; 1.0041x over previous
"""HNetLoss on 8 Trainium2 NeuronCores — v3 (fp8 DoubleRow + byte planes).

Math: per (batch, lane, row) the loss reduces to masked column moments
S_j[l, r] = sum_x w_j(x) [lab[r,x]==l] for w in {1, xc, xc^2} (xc=x-256);
the rest is exact host math (see _finalize).

Device scheme — five independent label functions, all vanishing at 0:
  * The bf16 label tile BITCAST to fp8 yields two FREE functions: the
    byte planes decode as f_lo(lab) = [0,-0,0,2,-0,-0.125] and
    f_hi(lab) = [0,1.875,2,2,2,2].
  * Three DVE is_equal masks (lanes 4,1,2); a bf16 1.0 mask's payload
    byte is fp8 1.875.
  Host solves the well-conditioned 5x5 system for per-lane moments.

PE: all contractions are fp8 MatmulPerfMode.DoubleRow (0.5 cyc/row).
TRN2 dual-fp8 ISA restrictions (reverse-engineered from neuronxcc):
exactly 16 stationary columns, k-tile weight stride 16, PSUM dst
partition 0 — so every DR matmul lands in PSUM rows 0:16.  Two moment
sets share each 256-col PSUM region via complementary zero-padded
stationary columns (set A rows 0:7, set B rows 8:15); the zero columns
write zeros, so no PSUM memsets are needed anywhere.

Input is split into two DMAs (chunk pair 0-1 + weights, then 2-3) so
mask/matmul work on the first half overlaps the second transfer.
Outputs: out1 [16, 512] = [lo|hi] rows 0:7, [m1|m2] rows 8:15;
out2 [7, 256] = m4.
"""

import sys

import numpy as np

try:
    import concourse.bass as bass  # noqa: F401
except ModuleNotFoundError:  # pragma: no cover
    sys.path.insert(0, "/opt/trn_rl_repo")

import ml_dtypes

import concourse.bacc as bacc
import concourse.bass as bass
import concourse.mybir as mybir
import concourse.tile as tile

ORDER = 3
N_LANES = 5
EPS_DEN = 1e-5
RIDGE = 1e-6

B, H, W = 8, 256, 512
N_CORES = 8
XC = 256.0
N_CHUNKS = W // 128

BF16 = mybir.dt.bfloat16
FP8 = mybir.dt.float8e4
F32 = mybir.dt.float32

LABW = N_CHUNKS * H  # 1024 label columns
NW8 = 7  # real fp8 weight columns: [1, xh, xl, ah, al, bh, bl]
MCOL = 16  # dual-fp8 ldweights requires exactly 16 stationary columns
ROWB = 8  # row offset of the second moment set within a PSUM region
WCOLS = 2 * N_CHUNKS * MCOL  # fp8 weight cols (A and B variants)
HALFW = LABW // 2  # label cols per input half (chunk pair)
INWA = HALFW + WCOLS // 2  # first half also carries the fp8 weights

MSCALE = 1.875  # payload byte of bf16 1.0 (0x3F80 -> 0x3F = 1.875)
CSCALE = 1.5984456304202803  # sc = bf16(CSCALE * lab): plane-diverse scaling


def _byte_planes(vals: np.ndarray):
    """fp8 e4m3 decodes of the (lo, hi) bytes of bf16(vals)."""
    bf = np.asarray(vals, dtype=ml_dtypes.bfloat16)
    by = bf.view(np.uint8).reshape(-1, 2)
    lo = by[:, 0].copy().view(ml_dtypes.float8_e4m3).astype(np.float64)
    hi = by[:, 1].copy().view(ml_dtypes.float8_e4m3).astype(np.float64)
    return lo, hi


def _unmix_matrix() -> np.ndarray:
    """5x5 map from per-lane moments to the five device functions.

    Function order: lab-lo-plane, lab-hi-plane, sc-lo-plane, sc-hi-plane,
    m1 (payload-scaled is_equal mask).  All vanish at lab=0.
    """
    lanes = np.arange(1, 6, dtype=np.float64)
    f_lo, f_hi = _byte_planes(lanes)
    sc = (np.float32(CSCALE) * lanes.astype(np.float32)).astype(np.float64)
    s_lo, s_hi = _byte_planes(sc)
    m1 = np.array([MSCALE, 0.0, 0.0, 0.0, 0.0])
    M = np.stack([f_lo, f_hi, s_lo, s_hi, m1])
    assert np.all(np.isfinite(M)) and abs(np.linalg.det(M)) > 1.0
    return M


def _build_program() -> bass.Bass:
    nc = bacc.Bacc("TRN2", target_bir_lowering=False)
    inpa_d = nc.declare_dram_parameter("inpa", [128, INWA], BF16, isOutput=False)
    inpb_d = nc.declare_dram_parameter("inpb", [128, HALFW], BF16, isOutput=False)
    out1_d = nc.declare_dram_parameter("out1", [16, 512], F32, isOutput=True)
    out2_d = nc.declare_dram_parameter("out2", [7, 256], F32, isOutput=True)

    with tile.TileContext(nc) as tc:
        with (
            tc.tile_pool(name="io", bufs=1) as io_pool,
            tc.tile_pool(name="masks", bufs=3) as mask_pool,
            tc.tile_pool(name="psum", bufs=2, space="PSUM") as psum_pool,
        ):
            inba = io_pool.tile([128, INWA], BF16, tag="inba")
            inbb = io_pool.tile([128, HALFW], BF16, tag="inbb")
            outba = io_pool.tile([128, 512], F32, tag="outba")
            outbb = io_pool.tile([128, 256], F32, tag="outbb")
            bank1 = psum_pool.tile([128, 512], F32, tag="bank1")
            bank2 = psum_pool.tile([128, 256], F32, tag="bank2")

            # Each bank runs ONE accumulation group (CoreSim's pending-zero
            # tracking is 2KB-row granular): only the first matmul sets
            # start, only the last sets stop, and the early memsets below
            # give the hardware zeros to accumulate onto for regions the
            # start-matmul does not touch.
            nc.vector.memset(bank1[:], 0.0)
            nc.vector.memset(bank2[:], 0.0)

            # split input: chunks 0-1 (+ weights) land ~600ns before 2-3,
            # so masks/matmuls on the first half overlap the second transfer.
            # Half B goes through the Pool-engine SWDGE path so its
            # descriptor generation runs in parallel with half A's HWDGE
            # ring instead of serializing behind it.
            nc.sync.dma_start(inba[:], inpa_d[:])
            nc.gpsimd.dma_start(inbb[:], inpb_d[:])

            labh = [inba[:, :HALFW], inbb[:]]  # chunk pairs (0,1), (2,3)
            # fp8 weights: per chunk MCOL cols variant A (w in cols 0:7)
            # then MCOL cols variant B (w in cols 8:15); [128, 2, 16] pairs
            wf8 = inba[:, HALFW:].bitcast(FP8)  # [128, 128]
            w4 = wf8.rearrange("p (c v j) -> p c v j", c=N_CHUNKS, v=2, j=MCOL)
            wA = [w4[:, 0:2, 0, :], w4[:, 2:4, 0, :]]
            wB = [w4[:, 0:2, 1, :], w4[:, 2:4, 1, :]]

            # DVE stream per half: sc = bf16(CSCALE * lab) (its two fp8 byte
            # planes are two more independent functions), then the single m1
            # is_equal mask.  Only 4 DVE ops total — the chain ends with
            # m1-h2, and bank1 (which feeds the big copy) closes one op
            # earlier on sc-h2.
            m1t = mask_pool.tile([128, LABW], BF16, tag="m1t")
            sct = mask_pool.tile([128, LABW], BF16, tag="sct")

            def half_slice(t, half):
                return t[:, HALFW * half : HALFW * (half + 1)]

            def sc_op(half):
                nc.vector.tensor_scalar(
                    half_slice(sct, half),
                    labh[half],
                    float(CSCALE),
                    None,
                    mybir.AluOpType.mult,
                )

            def m1_op(half):
                nc.vector.tensor_scalar(
                    half_slice(m1t, half),
                    labh[half],
                    1.0,
                    None,
                    mybir.AluOpType.is_equal,
                )

            # half-1's m1 before half-1's sc: bank2 (m1) closes mid-stream
            # and its small copy + Pool DMA tail overlaps bank1's
            sc_op(0)
            m1_op(0)
            m1_op(1)
            sc_op(1)

            DR = mybir.MatmulPerfMode.DoubleRow

            def plane(src_ap, s):
                v = src_ap.bitcast(FP8).rearrange(
                    "p (c r s) -> p c r s", c=2, r=H, s=2
                )
                return v[:, :, :, s]

            # Matmul schedule: (region, weight variant, moving, half, start,
            # stop).  Bank1 = [lab-lo + sc-lo | lab-hi + sc-hi] closes on
            # sc-hi-23; bank2 = m1 closes last but only feeds the small
            # copy + Pool-DMA tail that overlaps out1's HWDGE leg.
            sched = [
                (bank1[0:MCOL, 0:256], wA, plane(labh[0], 0), 0, True, False),
                (bank1[0:MCOL, 256:512], wA, plane(labh[0], 1), 0, False, False),
                (bank1[0:MCOL, 0:256], wB, plane(half_slice(sct, 0), 0), 0, False, False),
                (bank1[0:MCOL, 256:512], wB, plane(half_slice(sct, 0), 1), 0, False, False),
                (bank1[0:MCOL, 0:256], wA, plane(labh[1], 0), 1, False, False),
                (bank1[0:MCOL, 256:512], wA, plane(labh[1], 1), 1, False, False),
                (bank2[0:MCOL, 0:256], wA, plane(half_slice(m1t, 0), 1), 0, True, False),
                (bank2[0:MCOL, 0:256], wA, plane(half_slice(m1t, 1), 1), 1, False, True),
                (bank1[0:MCOL, 0:256], wB, plane(half_slice(sct, 1), 0), 1, False, False),
                (bank1[0:MCOL, 256:512], wB, plane(half_slice(sct, 1), 1), 1, False, True),
            ]
            for out_ap, w, moving, half, start, stop in sched:
                nc.tensor.matmul(
                    out_ap,
                    w[half],
                    moving,
                    start=start,
                    stop=stop,
                    perf_mode=DR,
                    tile_position=(0, 0),
                    skip_group_check=True,
                )

            # PSUM -> SBUF staging on separate tiles (no WAW between them).
            # m4 finishes early: stage it on Act and ship via the Pool SWDGE
            # so the critical bank1 copy (DVE) + its HWDGE DMA never wait.
            nc.scalar.copy(outbb[0:NW8, :], bank2[0:NW8, 0:256])
            nc.vector.tensor_copy(outba[0:MCOL, :], bank1[0:MCOL, :])

            nc.gpsimd.dma_start(out2_d[:], outbb[0:NW8, :])
            nc.sync.dma_start(out1_d[:], outba[0:MCOL, :])

    # Drop the framework's const-tile memsets from the entry block: nothing
    # in this program reads the const APs (all scalar operands are literal
    # immediates), and the 4 serial Pool memsets (95ns each) gate the entry
    # barrier and hence the input DMA issue.
    blk0 = nc.m.functions[0].blocks[0]
    dead = [
        i
        for i, inst in enumerate(blk0.instructions)
        if inst.opcode == "Memset"
        and any("const-" in str(o) for o in inst.outs)
    ]
    if dead:
        used = set()
        for fn in nc.m.functions:
            for blk in fn.blocks:
                for inst in blk.instructions:
                    if inst.opcode == "Memset":
                        continue
                    for ap in list(inst.ins) + list(inst.outs):
                        used.add(str(ap))
        assert not any("const-" in u for u in used), "const APs are used"
        for i in reversed(dead):
            del blk0.instructions[i]
    # With the memsets gone the entry all-engine barrier synchronizes idle
    # engines only; drop it too so the input DMA issues immediately.
    bar = [
        i
        for i, inst in enumerate(blk0.instructions)
        if inst.opcode in ("Drain", "EventSemaphore")
    ]
    for i in reversed(bar):
        del blk0.instructions[i]
    # Exit block: keep the SP queue-drain waits (output-DMA completion) but
    # drop the two all-engine barrier ping-pong rounds and the semaphore
    # range clear — they only matter for re-executing a still-loaded NEFF.
    blk2 = nc.m.functions[0].blocks[2]
    tail = [
        i
        for i, inst in enumerate(blk2.instructions)
        if "barrier" in inst.concise()
        or "EVENT_SEMAPHORE_RANGE_CLEAR" in inst.concise()
        or (inst.opcode == "Drain" and "is_reset_sema=True" in inst.concise())
    ]
    for i in reversed(tail):
        del blk2.instructions[i]
    nc.compile()
    return nc


def _fp8_hi_lo(vals: np.ndarray, clip: float = 240.0):
    """Exact v = hi + lo split with both parts fp8 e4m3 (max normal 240)."""
    e4 = ml_dtypes.float8_e4m3
    hi = np.clip(vals, -clip, clip).astype(e4)
    hi64 = hi.astype(np.float64)
    lo = (vals - hi64).astype(e4)
    assert np.all(lo.astype(np.float64) + hi64 == vals), "fp8 split not exact"
    return hi, lo


def _fp8_weights() -> np.ndarray:
    """[128, WCOLS] fp8: per chunk, 16 cols variant A ([1,xh,xl,ah,al,bh,bl]
    in cols 0:7) then 16 cols variant B (same weights in cols 8:15)."""
    e4 = ml_dtypes.float8_e4m3
    x = np.arange(W, dtype=np.float64)
    xc = x - XC
    xc2 = xc * xc
    a = np.floor(xc2 / 256.0)
    b = xc2 - 256.0 * a
    xh, xl = _fp8_hi_lo(xc)
    ah, al = _fp8_hi_lo(a)
    bh, bl = _fp8_hi_lo(b)
    wreal = np.zeros((W, NW8), dtype=e4)
    wreal[:, 0] = 1.0
    wreal[:, 1] = xh
    wreal[:, 2] = xl
    wreal[:, 3] = ah
    wreal[:, 4] = al
    wreal[:, 5] = bh
    wreal[:, 6] = bl
    w = np.zeros((W, 2, MCOL), dtype=e4)
    w[:, 0, :NW8] = wreal
    w[:, 1, ROWB : ROWB + NW8] = wreal
    # [x, v, j] -> [p, c, v, j] -> [128, WCOLS]
    return np.ascontiguousarray(
        w.reshape(N_CHUNKS, 128, 2 * MCOL).transpose(1, 0, 2).reshape(128, WCOLS)
    )


def _host_prep(instance_label: np.ndarray):
    lab = np.asarray(instance_label)
    wf8 = _fp8_weights()
    in_maps = []
    for b in range(B):
        lt = lab[b].T.astype(ml_dtypes.bfloat16)  # [W, H]
        lt = lt.reshape(N_CHUNKS, 128, H).transpose(1, 0, 2).reshape(128, LABW)
        inpa = np.empty((128, INWA), dtype=ml_dtypes.bfloat16)
        inpa[:, :HALFW] = lt[:, :HALFW]
        inpa[:, HALFW:].view(np.uint8)[:] = wf8.view(np.uint8)
        in_maps.append({"inpa": inpa, "inpb": np.ascontiguousarray(lt[:, HALFW:])})
    return in_maps


def _decode_moments(raw1: np.ndarray, raw2: np.ndarray) -> np.ndarray:
    """Device outputs -> per-lane moments [3, N_LANES, H] f64.

    raw1 [16, 512]: rows 0:7 = [lab-lo | lab-hi], rows 8:15 =
    [sc-lo | sc-hi]; raw2 [7, 256]: m1 (payload-scaled).
    Moment index: 0 = count, 1 = sum xc, 2 = sum xc^2.
    """
    g1 = raw1.astype(np.float64)
    g2 = raw2.astype(np.float64)

    def comb(t):  # [7, H] fp8-moment rows -> [3, H]
        return np.stack(
            [t[0], t[1] + t[2], 256.0 * (t[3] + t[4]) + t[5] + t[6]]
        )

    t = np.stack(
        [
            comb(g1[0:NW8, 0:256]),  # lab lo plane
            comb(g1[0:NW8, 256:512]),  # lab hi plane
            comb(g1[ROWB : ROWB + NW8, 0:256]),  # sc lo plane
            comb(g1[ROWB : ROWB + NW8, 256:512]),  # sc hi plane
            comb(g2),  # m1 (payload-scaled)
        ]
    )  # [5 funcs, 3 moments, H]
    u = np.linalg.solve(_unmix_matrix(), t.reshape(5, -1)).reshape(t.shape)
    return u.transpose(1, 0, 2)  # [3, L, H]


def _finalize(hnet_params: np.ndarray, moments: np.ndarray) -> np.float32:
    """moments: [B, 3, L, H] f64 (count, S1 about XC, S2 about XC)."""
    p = np.asarray(hnet_params, dtype=np.float64)
    c = moments[:, 0]  # [B, L, H]
    S1c = moments[:, 1]
    S2c = moments[:, 2]
    S1 = S1c + XC * c
    S2 = S2c + 2.0 * XC * S1c + XC * XC * c

    r = np.arange(H, dtype=np.float64)
    p32 = np.asarray(hnet_params, dtype=np.float32)
    den32 = (p32[:, 5:6] * r.astype(np.float32)[None, :]) + np.float32(1.0)
    den = np.where(np.abs(den32) < EPS_DEN, np.float32(EPS_DEN), den32).astype(
        np.float64
    )
    alpha = p[:, 0:1] / den  # [B,H]
    beta = (p[:, 1:2] * r[None, :] + p[:, 2:3]) / den
    yp = (p[:, 3:4] * r[None, :] + p[:, 4:5]) / den

    al = alpha[:, None, :]
    be = beta[:, None, :]
    Sx = al * S1 + be * c
    Sxx = al * al * S2 + 2 * al * be * S1 + be * be * c

    ypb = yp[:, None, :]
    cnt = c.sum(-1)  # [B,L]
    s = np.stack([(c * ypb**k).sum(-1) for k in range(7)], axis=-1)
    t = np.stack([(Sx * ypb**q).sum(-1) for q in range(4)], axis=-1)
    v = (c * np.abs(den)[:, None, :]).sum(-1)

    k = ORDER + 1
    A0 = np.empty((B, N_LANES, k, k))
    for i in range(k):
        for j in range(k):
            A0[:, :, i, j] = s[:, :, 6 - i - j]
    rhs = np.stack([t[:, :, 3 - i] for i in range(k)], axis=-1)
    A = A0 + RIDGE * np.eye(k)
    w = np.linalg.solve(A, rhs[..., None])[..., 0]

    xpred = sum(w[:, :, i, None] * ypb ** (3 - i) for i in range(k))
    rss = (Sxx - 2 * xpred * Sx + xpred * xpred * c).sum(-1)

    cnt_safe = np.maximum(cnt, 1.0)
    lane_loss = (rss / cnt_safe) * (v / cnt_safe)
    valid = (cnt >= ORDER + 1).astype(np.float64)
    nv = valid.sum()
    loss = (valid * lane_loss).sum() / max(nv, 1.0) if nv > 0 else 0.0
    return np.float32(loss)


def _run_device(in_maps, trace: bool = False, trace_cores=None):
    from concourse import bass_utils

    nc = _build_program()
    res = bass_utils.run_bass_kernel_spmd(
        nc,
        in_maps,
        core_ids=list(range(N_CORES)),
        trace=trace,
        trace_cores=trace_cores,
    )
    return res


def kernel(hnet_params: np.ndarray, instance_label: np.ndarray) -> np.ndarray:
    in_maps = _host_prep(instance_label)
    res = _run_device(in_maps)
    moments = np.stack(
        [
            _decode_moments(
                np.asarray(res.results[b]["out1"]),
                np.asarray(res.results[b]["out2"]),
            )
            for b in range(B)
        ]
    )
    return _finalize(hnet_params, moments)


def _golden_moments(lab_b: np.ndarray) -> np.ndarray:
    """Numpy golden for one batch: [3, L, H] exact moments."""
    x = np.arange(W, dtype=np.float64)
    xc = x - XC
    out = np.zeros((3, N_LANES, H))
    for lane in range(N_LANES):
        msk = lab_b == (lane + 1)  # [H, W]
        out[0, lane] = msk.sum(1)
        out[1, lane] = (msk * xc).sum(1)
        out[2, lane] = (msk * xc * xc).sum(1)
    return out


if __name__ == "__main__":
    from concourse.bass_interp import CoreSim

    rng = np.random.default_rng(0)
    lab_full = rng.integers(0, 6, size=(B, H, W)).astype(np.int64)
    in_maps = _host_prep(lab_full)

    nc = _build_program()
    sim = CoreSim(nc)
    sim.tensor("inpa")[:] = in_maps[0]["inpa"]
    sim.tensor("inpb")[:] = in_maps[0]["inpb"]
    sim.simulate()
    mom = _decode_moments(
        np.asarray(sim.tensor("out1")), np.asarray(sim.tensor("out2"))
    )

    golden = _golden_moments(lab_full[0])
    err = np.abs(mom - golden)
    rel = err.max() / max(np.abs(golden).max(), 1)
    print("max abs err:", err.max(), "max rel:", rel)
    assert rel < 1e-6, "CoreSim moments mismatch"
    print("CoreSim moments check PASSED")


# revision 6
# speedup vs baseline: 1.8339x; 1.0153x over previous
"""HNetLoss on 8 Trainium2 NeuronCores — v3 (fp8 DoubleRow + byte planes).

Math: per (batch, lane, row) the loss reduces to masked column moments
S_j[l, r] = sum_x w_j(x) [lab[r,x]==l] for w in {1, xc, xc^2} (xc=x-256);
the rest is exact host math (see _finalize).

Device scheme — five independent label functions, all vanishing at 0:
  * The bf16 label tile BITCAST to fp8 yields two FREE functions: the
    byte planes decode as f_lo(lab) = [0,-0,0,2,-0,-0.125] and
    f_hi(lab) = [0,1.875,2,2,2,2].
  * Three DVE is_equal masks (lanes 4,1,2); a bf16 1.0 mask's payload
    byte is fp8 1.875.
  Host solves the well-conditioned 5x5 system for per-lane moments.

PE: all contractions are fp8 MatmulPerfMode.DoubleRow (0.5 cyc/row).
TRN2 dual-fp8 ISA restrictions (reverse-engineered from neuronxcc):
exactly 16 stationary columns, k-tile weight stride 16, PSUM dst
partition 0 — so every DR matmul lands in PSUM rows 0:16.  Two moment
sets share each 256-col PSUM region via complementary zero-padded
stationary columns (set A rows 0:7, set B rows 8:15); the zero columns
write zeros, so no PSUM memsets are needed anywhere.

Input is split into two DMAs (chunk pair 0-1 + weights, then 2-3) so
mask/matmul work on the first half overlaps the second transfer.
Outputs: out1 [16, 512] = [lo|hi] rows 0:7, [m1|m2] rows 8:15;
out2 [7, 256] = m4.
"""

import sys

import numpy as np

try:
    import concourse.bass as bass  # noqa: F401
except ModuleNotFoundError:  # pragma: no cover
    sys.path.insert(0, "/opt/trn_rl_repo")

import ml_dtypes

import concourse.bacc as bacc
import concourse.bass as bass
import concourse.mybir as mybir
import concourse.tile as tile

ORDER = 3
N_LANES = 5
EPS_DEN = 1e-5
RIDGE = 1e-6

B, H, W = 8, 256, 512
N_CORES = 8
XC = 256.0
N_CHUNKS = W // 128

BF16 = mybir.dt.bfloat16
FP8 = mybir.dt.float8e4
F32 = mybir.dt.float32

LABW = N_CHUNKS * H  # 1024 label columns
NW8 = 7  # real fp8 weight columns: [1, xh, xl, ah, al, bh, bl]
MCOL = 16  # dual-fp8 ldweights requires exactly 16 stationary columns
ROWB = 8  # row offset of the second moment set within a PSUM region
WCOLS = 2 * N_CHUNKS * MCOL  # fp8 weight cols (A and B variants)
HALFW = LABW // 2  # label cols per input half (chunk pair)
INWA = HALFW + WCOLS // 2  # first half also carries the fp8 weights

MSCALE = 1.875  # payload byte of bf16 1.0 (0x3F80 -> 0x3F = 1.875)
CSCALE = 1.5984456304202803  # sc = bf16(CSCALE * lab): plane-diverse scaling


def _byte_planes(vals: np.ndarray):
    """fp8 e4m3 decodes of the (lo, hi) bytes of bf16(vals)."""
    bf = np.asarray(vals, dtype=ml_dtypes.bfloat16)
    by = bf.view(np.uint8).reshape(-1, 2)
    lo = by[:, 0].copy().view(ml_dtypes.float8_e4m3).astype(np.float64)
    hi = by[:, 1].copy().view(ml_dtypes.float8_e4m3).astype(np.float64)
    return lo, hi


def _unmix_matrix() -> np.ndarray:
    """5x5 map from per-lane moments to the five device functions.

    Function order: lab-lo-plane, lab-hi-plane, sc-lo-plane, sc-hi-plane,
    m1 (payload-scaled is_equal mask).  All vanish at lab=0.
    """
    lanes = np.arange(1, 6, dtype=np.float64)
    f_lo, f_hi = _byte_planes(lanes)
    sc = (np.float32(CSCALE) * lanes.astype(np.float32)).astype(np.float64)
    s_lo, s_hi = _byte_planes(sc)
    m1 = np.array([MSCALE, 0.0, 0.0, 0.0, 0.0])
    M = np.stack([f_lo, f_hi, s_lo, s_hi, m1])
    assert np.all(np.isfinite(M)) and abs(np.linalg.det(M)) > 1.0
    return M


def _build_program() -> bass.Bass:
    nc = bacc.Bacc("TRN2", target_bir_lowering=False)
    inpa_d = nc.declare_dram_parameter("inpa", [128, INWA], BF16, isOutput=False)
    inpb_d = nc.declare_dram_parameter("inpb", [128, HALFW], BF16, isOutput=False)
    out1_d = nc.declare_dram_parameter("out1", [16, 512], F32, isOutput=True)
    out2_d = nc.declare_dram_parameter("out2", [7, 256], F32, isOutput=True)

    with tile.TileContext(nc) as tc:
        with (
            tc.tile_pool(name="io", bufs=1) as io_pool,
            tc.tile_pool(name="masks", bufs=3) as mask_pool,
            tc.tile_pool(name="psum", bufs=2, space="PSUM") as psum_pool,
        ):
            inba = io_pool.tile([128, INWA], BF16, tag="inba")
            inbb = io_pool.tile([128, HALFW], BF16, tag="inbb")
            outba = io_pool.tile([128, 512], F32, tag="outba")
            outbb = io_pool.tile([128, 256], F32, tag="outbb")
            bank1 = psum_pool.tile([128, 512], F32, tag="bank1")
            bank2 = psum_pool.tile([128, 256], F32, tag="bank2")

            # Each bank runs ONE accumulation group (CoreSim's pending-zero
            # tracking is 2KB-row granular): only the first matmul sets
            # start, only the last sets stop, and the early memsets below
            # give the hardware zeros to accumulate onto for regions the
            # start-matmul does not touch.
            nc.vector.memset(bank1[:], 0.0)
            nc.vector.memset(bank2[:], 0.0)

            # split input: chunks 0-1 (+ weights) land ~600ns before 2-3,
            # so masks/matmuls on the first half overlap the second transfer.
            # Half B goes through the Pool-engine SWDGE path so its
            # descriptor generation runs in parallel with half A's HWDGE
            # ring instead of serializing behind it.
            nc.sync.dma_start(inba[:], inpa_d[:])
            nc.gpsimd.dma_start(inbb[:], inpb_d[:])

            labh = [inba[:, :HALFW], inbb[:]]  # chunk pairs (0,1), (2,3)
            # fp8 weights: per chunk MCOL cols variant A (w in cols 0:7)
            # then MCOL cols variant B (w in cols 8:15); [128, 2, 16] pairs
            wf8 = inba[:, HALFW:].bitcast(FP8)  # [128, 128]
            w4 = wf8.rearrange("p (c v j) -> p c v j", c=N_CHUNKS, v=2, j=MCOL)
            wA = [w4[:, 0:2, 0, :], w4[:, 2:4, 0, :]]
            wB = [w4[:, 0:2, 1, :], w4[:, 2:4, 1, :]]

            # DVE stream per half: sc = bf16(CSCALE * lab) (its two fp8 byte
            # planes are two more independent functions), then the single m1
            # is_equal mask.  Only 4 DVE ops total — the chain ends with
            # m1-h2, and bank1 (which feeds the big copy) closes one op
            # earlier on sc-h2.
            m1t = mask_pool.tile([128, LABW], BF16, tag="m1t")
            sct = mask_pool.tile([128, LABW], BF16, tag="sct")

            def half_slice(t, half):
                return t[:, HALFW * half : HALFW * (half + 1)]

            def sc_op(half):
                nc.vector.tensor_scalar(
                    half_slice(sct, half),
                    labh[half],
                    float(CSCALE),
                    None,
                    mybir.AluOpType.mult,
                )

            def m1_op(half):
                nc.vector.tensor_scalar(
                    half_slice(m1t, half),
                    labh[half],
                    1.0,
                    None,
                    mybir.AluOpType.is_equal,
                )

            # half-1's m1 before half-1's sc: bank2 (m1) closes mid-stream
            # and its small copy + Pool DMA tail overlaps bank1's
            sc_op(0)
            m1_op(0)
            m1_op(1)
            sc_op(1)

            DR = mybir.MatmulPerfMode.DoubleRow

            def plane(src_ap, s):
                v = src_ap.bitcast(FP8).rearrange(
                    "p (c r s) -> p c r s", c=2, r=H, s=2
                )
                return v[:, :, :, s]

            # Matmul schedule: (region, weight variant, moving, half, start,
            # stop).  Bank1 = [lab-lo + sc-lo | lab-hi + sc-hi] closes on
            # sc-hi-23; bank2 = m1 closes last but only feeds the small
            # copy + Pool-DMA tail that overlaps out1's HWDGE leg.
            sched = [
                (bank1[0:MCOL, 0:256], wA, plane(labh[0], 0), 0, True, False),
                (bank1[0:MCOL, 256:512], wA, plane(labh[0], 1), 0, False, False),
                (bank1[0:MCOL, 0:256], wB, plane(half_slice(sct, 0), 0), 0, False, False),
                (bank1[0:MCOL, 256:512], wB, plane(half_slice(sct, 0), 1), 0, False, False),
                (bank2[0:MCOL, 0:256], wA, plane(half_slice(m1t, 0), 1), 0, True, False),
                (bank1[0:MCOL, 0:256], wA, plane(labh[1], 0), 1, False, False),
                (bank2[0:MCOL, 0:256], wA, plane(half_slice(m1t, 1), 1), 1, False, True),
                (bank1[0:MCOL, 256:512], wA, plane(labh[1], 1), 1, False, False),
                (bank1[0:MCOL, 0:256], wB, plane(half_slice(sct, 1), 0), 1, False, False),
                (bank1[0:MCOL, 256:512], wB, plane(half_slice(sct, 1), 1), 1, False, True),
            ]
            for out_ap, w, moving, half, start, stop in sched:
                nc.tensor.matmul(
                    out_ap,
                    w[half],
                    moving,
                    start=start,
                    stop=stop,
                    perf_mode=DR,
                    tile_position=(0, 0),
                    skip_group_check=True,
                )

            # PSUM -> SBUF staging on separate tiles (no WAW between them).
            # m4 finishes early: stage it on Act and ship via the Pool SWDGE
            # so the critical bank1 copy (DVE) + its HWDGE DMA never wait.
            nc.scalar.copy(outbb[0:NW8, :], bank2[0:NW8, 0:256])
            nc.vector.tensor_copy(outba[0:MCOL, :], bank1[0:MCOL, :])

            nc.gpsimd.dma_start(out2_d[:], outbb[0:NW8, :])
            nc.sync.dma_start(out1_d[:], outba[0:MCOL, :])

    # Drop the framework's const-tile memsets from the entry block: nothing
    # in this program reads the const APs (all scalar operands are literal
    # immediates), and the 4 serial Pool memsets (95ns each) gate the entry
    # barrier and hence the input DMA issue.
    blk0 = nc.m.functions[0].blocks[0]
    dead = [
        i
        for i, inst in enumerate(blk0.instructions)
        if inst.opcode == "Memset"
        and any("const-" in str(o) for o in inst.outs)
    ]
    if dead:
        used = set()
        for fn in nc.m.functions:
            for blk in fn.blocks:
                for inst in blk.instructions:
                    if inst.opcode == "Memset":
                        continue
                    for ap in list(inst.ins) + list(inst.outs):
                        used.add(str(ap))
        assert not any("const-" in u for u in used), "const APs are used"
        for i in reversed(dead):
            del blk0.instructions[i]
    # With the memsets gone the entry all-engine barrier synchronizes idle
    # engines only; drop it too so the input DMA issues immediately.
    bar = [
        i
        for i, inst in enumerate(blk0.instructions)
        if inst.opcode in ("Drain", "EventSemaphore")
    ]
    for i in reversed(bar):
        del blk0.instructions[i]
    # Exit block: keep the SP queue-drain waits (output-DMA completion) but
    # drop the two all-engine barrier ping-pong rounds and the semaphore
    # range clear — they only matter for re-executing a still-loaded NEFF.
    blk2 = nc.m.functions[0].blocks[2]
    tail = [
        i
        for i, inst in enumerate(blk2.instructions)
        if "barrier" in inst.concise()
        or "EVENT_SEMAPHORE_RANGE_CLEAR" in inst.concise()
        or (inst.opcode == "Drain" and "is_reset_sema=True" in inst.concise())
    ]
    for i in reversed(tail):
        del blk2.instructions[i]
    sp_waits = [
        i
        for i, inst in enumerate(blk2.instructions)
        if inst.opcode == "EventSemaphore" and "DMASW" in inst.concise()
    ]
    for k, i in enumerate(sp_waits):
        if i != k:
            inst = blk2.instructions.pop(i)
            blk2.instructions.insert(k, inst)
    nc.compile()
    return nc


def _fp8_hi_lo(vals: np.ndarray, clip: float = 240.0):
    """Exact v = hi + lo split with both parts fp8 e4m3 (max normal 240)."""
    e4 = ml_dtypes.float8_e4m3
    hi = np.clip(vals, -clip, clip).astype(e4)
    hi64 = hi.astype(np.float64)
    lo = (vals - hi64).astype(e4)
    assert np.all(lo.astype(np.float64) + hi64 == vals), "fp8 split not exact"
    return hi, lo


def _fp8_weights() -> np.ndarray:
    """[128, WCOLS] fp8: per chunk, 16 cols variant A ([1,xh,xl,ah,al,bh,bl]
    in cols 0:7) then 16 cols variant B (same weights in cols 8:15)."""
    e4 = ml_dtypes.float8_e4m3
    x = np.arange(W, dtype=np.float64)
    xc = x - XC
    xc2 = xc * xc
    a = np.floor(xc2 / 256.0)
    b = xc2 - 256.0 * a
    xh, xl = _fp8_hi_lo(xc)
    ah, al = _fp8_hi_lo(a)
    bh, bl = _fp8_hi_lo(b)
    wreal = np.zeros((W, NW8), dtype=e4)
    wreal[:, 0] = 1.0
    wreal[:, 1] = xh
    wreal[:, 2] = xl
    wreal[:, 3] = ah
    wreal[:, 4] = al
    wreal[:, 5] = bh
    wreal[:, 6] = bl
    w = np.zeros((W, 2, MCOL), dtype=e4)
    w[:, 0, :NW8] = wreal
    w[:, 1, ROWB : ROWB + NW8] = wreal
    # [x, v, j] -> [p, c, v, j] -> [128, WCOLS]
    return np.ascontiguousarray(
        w.reshape(N_CHUNKS, 128, 2 * MCOL).transpose(1, 0, 2).reshape(128, WCOLS)
    )


def _host_prep(instance_label: np.ndarray):
    lab = np.asarray(instance_label)
    wf8 = _fp8_weights()
    in_maps = []
    for b in range(B):
        lt = lab[b].T.astype(ml_dtypes.bfloat16)  # [W, H]
        lt = lt.reshape(N_CHUNKS, 128, H).transpose(1, 0, 2).reshape(128, LABW)
        inpa = np.empty((128, INWA), dtype=ml_dtypes.bfloat16)
        inpa[:, :HALFW] = lt[:, :HALFW]
        inpa[:, HALFW:].view(np.uint8)[:] = wf8.view(np.uint8)
        in_maps.append({"inpa": inpa, "inpb": np.ascontiguousarray(lt[:, HALFW:])})
    return in_maps


def _decode_moments(raw1: np.ndarray, raw2: np.ndarray) -> np.ndarray:
    """Device outputs -> per-lane moments [3, N_LANES, H] f64.

    raw1 [16, 512]: rows 0:7 = [lab-lo | lab-hi], rows 8:15 =
    [sc-lo | sc-hi]; raw2 [7, 256]: m1 (payload-scaled).
    Moment index: 0 = count, 1 = sum xc, 2 = sum xc^2.
    """
    g1 = raw1.astype(np.float64)
    g2 = raw2.astype(np.float64)

    def comb(t):  # [7, H] fp8-moment rows -> [3, H]
        return np.stack(
            [t[0], t[1] + t[2], 256.0 * (t[3] + t[4]) + t[5] + t[6]]
        )

    t = np.stack(
        [
            comb(g1[0:NW8, 0:256]),  # lab lo plane
            comb(g1[0:NW8, 256:512]),  # lab hi plane
            comb(g1[ROWB : ROWB + NW8, 0:256]),  # sc lo plane
            comb(g1[ROWB : ROWB + NW8, 256:512]),  # sc hi plane
            comb(g2),  # m1 (payload-scaled)
        ]
    )  # [5 funcs, 3 moments, H]
    u = np.linalg.solve(_unmix_matrix(), t.reshape(5, -1)).reshape(t.shape)
    return u.transpose(1, 0, 2)  # [3, L, H]


def _finalize(hnet_params: np.ndarray, moments: np.ndarray) -> np.float32:
    """moments: [B, 3, L, H] f64 (count, S1 about XC, S2 about XC)."""
    p = np.asarray(hnet_params, dtype=np.float64)
    c = moments[:, 0]  # [B, L, H]
    S1c = moments[:, 1]
    S2c = moments[:, 2]
    S1 = S1c + XC * c
    S2 = S2c + 2.0 * XC * S1c + XC * XC * c

    r = np.arange(H, dtype=np.float64)
    p32 = np.asarray(hnet_params, dtype=np.float32)
    den32 = (p32[:, 5:6] * r.astype(np.float32)[None, :]) + np.float32(1.0)
    den = np.where(np.abs(den32) < EPS_DEN, np.float32(EPS_DEN), den32).astype(
        np.float64
    )
    alpha = p[:, 0:1] / den  # [B,H]
    beta = (p[:, 1:2] * r[None, :] + p[:, 2:3]) / den
    yp = (p[:, 3:4] * r[None, :] + p[:, 4:5]) / den

    al = alpha[:, None, :]
    be = beta[:, None, :]
    Sx = al * S1 + be * c
    Sxx = al * al * S2 + 2 * al * be * S1 + be * be * c

    ypb = yp[:, None, :]
    cnt = c.sum(-1)  # [B,L]
    s = np.stack([(c * ypb**k).sum(-1) for k in range(7)], axis=-1)
    t = np.stack([(Sx * ypb**q).sum(-1) for q in range(4)], axis=-1)
    v = (c * np.abs(den)[:, None, :]).sum(-1)

    k = ORDER + 1
    A0 = np.empty((B, N_LANES, k, k))
    for i in range(k):
        for j in range(k):
            A0[:, :, i, j] = s[:, :, 6 - i - j]
    rhs = np.stack([t[:, :, 3 - i] for i in range(k)], axis=-1)
    A = A0 + RIDGE * np.eye(k)
    w = np.linalg.solve(A, rhs[..., None])[..., 0]

    xpred = sum(w[:, :, i, None] * ypb ** (3 - i) for i in range(k))
    rss = (Sxx - 2 * xpred * Sx + xpred * xpred * c).sum(-1)

    cnt_safe = np.maximum(cnt, 1.0)
    lane_loss = (rss / cnt_safe) * (v / cnt_safe)
    valid = (cnt >= ORDER + 1).astype(np.float64)
    nv = valid.sum()
    loss = (valid * lane_loss).sum() / max(nv, 1.0) if nv > 0 else 0.0
    return np.float32(loss)


def _run_device(in_maps, trace: bool = False, trace_cores=None):
    from concourse import bass_utils

    nc = _build_program()
    res = bass_utils.run_bass_kernel_spmd(
        nc,
        in_maps,
        core_ids=list(range(N_CORES)),
        trace=trace,
        trace_cores=trace_cores,
    )
    return res


def kernel(hnet_params: np.ndarray, instance_label: np.ndarray) -> np.ndarray:
    in_maps = _host_prep(instance_label)
    res = _run_device(in_maps)
    moments = np.stack(
        [
            _decode_moments(
                np.asarray(res.results[b]["out1"]),
                np.asarray(res.results[b]["out2"]),
            )
            for b in range(B)
        ]
    )
    return _finalize(hnet_params, moments)


def _golden_moments(lab_b: np.ndarray) -> np.ndarray:
    """Numpy golden for one batch: [3, L, H] exact moments."""
    x = np.arange(W, dtype=np.float64)
    xc = x - XC
    out = np.zeros((3, N_LANES, H))
    for lane in range(N_LANES):
        msk = lab_b == (lane + 1)  # [H, W]
        out[0, lane] = msk.sum(1)
        out[1, lane] = (msk * xc).sum(1)
        out[2, lane] = (msk * xc * xc).sum(1)
    return out


if __name__ == "__main__":
    from concourse.bass_interp import CoreSim

    rng = np.random.default_rng(0)
    lab_full = rng.integers(0, 6, size=(B, H, W)).astype(np.int64)
    in_maps = _host_prep(lab_full)

    nc = _build_program()
    sim = CoreSim(nc)
    sim.tensor("inpa")[:] = in_maps[0]["inpa"]
    sim.tensor("inpb")[:] = in_maps[0]["inpb"]
    sim.simulate()
    mom = _decode_moments(
        np.asarray(sim.tensor("out1")), np.asarray(sim.tensor("out2"))
    )

    golden = _golden_moments(lab_full[0])
    err = np.abs(mom - golden)
    rel = err.max() / max(np.abs(golden).max(), 1)
    print("max abs err:", err.max(), "max rel:", rel)
    assert rel < 1e-6, "CoreSim moments mismatch"
    print("CoreSim moments check PASSED")


# revision 7
# speedup vs baseline: 1.8376x; 1.0020x over previous
"""HNetLoss on 8 Trainium2 NeuronCores — v3 (fp8 DoubleRow + byte planes).

Math: per (batch, lane, row) the loss reduces to masked column moments
S_j[l, r] = sum_x w_j(x) [lab[r,x]==l] for w in {1, xc, xc^2} (xc=x-256);
the rest is exact host math (see _finalize).

Device scheme — five independent label functions, all vanishing at 0:
  * The bf16 label tile BITCAST to fp8 yields two FREE functions: the
    byte planes decode as f_lo(lab) = [0,-0,0,2,-0,-0.125] and
    f_hi(lab) = [0,1.875,2,2,2,2].
  * Three DVE is_equal masks (lanes 4,1,2); a bf16 1.0 mask's payload
    byte is fp8 1.875.
  Host solves the well-conditioned 5x5 system for per-lane moments.

PE: all contractions are fp8 MatmulPerfMode.DoubleRow (0.5 cyc/row).
TRN2 dual-fp8 ISA restrictions (reverse-engineered from neuronxcc):
exactly 16 stationary columns, k-tile weight stride 16, PSUM dst
partition 0 — so every DR matmul lands in PSUM rows 0:16.  Two moment
sets share each 256-col PSUM region via complementary zero-padded
stationary columns (set A rows 0:7, set B rows 8:15); the zero columns
write zeros, so no PSUM memsets are needed anywhere.

Input is split into two DMAs (chunk pair 0-1 + weights, then 2-3) so
mask/matmul work on the first half overlaps the second transfer.
Outputs: out1 [16, 512] = [lo|hi] rows 0:7, [m1|m2] rows 8:15;
out2 [7, 256] = m4.
"""

import sys

import numpy as np

try:
    import concourse.bass as bass  # noqa: F401
except ModuleNotFoundError:  # pragma: no cover
    sys.path.insert(0, "/opt/trn_rl_repo")

import ml_dtypes

import concourse.bacc as bacc
import concourse.bass as bass
import concourse.mybir as mybir
import concourse.tile as tile

ORDER = 3
N_LANES = 5
EPS_DEN = 1e-5
RIDGE = 1e-6

B, H, W = 8, 256, 512
N_CORES = 8
XC = 256.0
N_CHUNKS = W // 128

BF16 = mybir.dt.bfloat16
FP8 = mybir.dt.float8e4
F32 = mybir.dt.float32

LABW = N_CHUNKS * H  # 1024 label columns
NW8 = 7  # real fp8 weight columns: [1, xh, xl, ah, al, bh, bl]
MCOL = 16  # dual-fp8 ldweights requires exactly 16 stationary columns
ROWB = 8  # row offset of the second moment set within a PSUM region
WCOLS = 8 + N_CHUNKS * MCOL  # fp8 weight cols (shared A/B table, see below)
HALFW = LABW // 2  # label cols per input half (chunk pair)
INWA = HALFW + WCOLS // 2  # first half also carries the fp8 weights

MSCALE = 1.875  # payload byte of bf16 1.0 (0x3F80 -> 0x3F = 1.875)
CSCALE = 1.5984456304202803  # sc = bf16(CSCALE * lab): plane-diverse scaling


def _byte_planes(vals: np.ndarray):
    """fp8 e4m3 decodes of the (lo, hi) bytes of bf16(vals)."""
    bf = np.asarray(vals, dtype=ml_dtypes.bfloat16)
    by = bf.view(np.uint8).reshape(-1, 2)
    lo = by[:, 0].copy().view(ml_dtypes.float8_e4m3).astype(np.float64)
    hi = by[:, 1].copy().view(ml_dtypes.float8_e4m3).astype(np.float64)
    return lo, hi


def _unmix_matrix() -> np.ndarray:
    """5x5 map from per-lane moments to the five device functions.

    Function order: lab-lo-plane, lab-hi-plane, sc-lo-plane, sc-hi-plane,
    m1 (payload-scaled is_equal mask).  All vanish at lab=0.
    """
    lanes = np.arange(1, 6, dtype=np.float64)
    f_lo, f_hi = _byte_planes(lanes)
    sc = (np.float32(CSCALE) * lanes.astype(np.float32)).astype(np.float64)
    s_lo, s_hi = _byte_planes(sc)
    m1 = np.array([MSCALE, 0.0, 0.0, 0.0, 0.0])
    M = np.stack([f_lo, f_hi, s_lo, s_hi, m1])
    assert np.all(np.isfinite(M)) and abs(np.linalg.det(M)) > 1.0
    return M


def _build_program() -> bass.Bass:
    nc = bacc.Bacc("TRN2", target_bir_lowering=False)
    inpa_d = nc.declare_dram_parameter("inpa", [128, INWA], BF16, isOutput=False)
    inpb_d = nc.declare_dram_parameter("inpb", [128, HALFW], BF16, isOutput=False)
    out1_d = nc.declare_dram_parameter("out1", [16, 512], F32, isOutput=True)
    out2_d = nc.declare_dram_parameter("out2", [7, 256], F32, isOutput=True)

    with tile.TileContext(nc) as tc:
        with (
            tc.tile_pool(name="io", bufs=1) as io_pool,
            tc.tile_pool(name="masks", bufs=3) as mask_pool,
            tc.tile_pool(name="psum", bufs=2, space="PSUM") as psum_pool,
        ):
            inba = io_pool.tile([128, INWA], BF16, tag="inba")
            inbb = io_pool.tile([128, HALFW], BF16, tag="inbb")
            outba = io_pool.tile([128, 512], F32, tag="outba")
            outbb = io_pool.tile([128, 256], F32, tag="outbb")
            bank1 = psum_pool.tile([128, 512], F32, tag="bank1")
            bank2 = psum_pool.tile([128, 256], F32, tag="bank2")

            # Each bank runs ONE accumulation group (CoreSim's pending-zero
            # tracking is 2KB-row granular): only the first matmul sets
            # start, only the last sets stop, and the early memsets below
            # give the hardware zeros to accumulate onto for regions the
            # start-matmul does not touch.
            nc.vector.memset(bank1[:], 0.0)
            nc.vector.memset(bank2[:], 0.0)

            # split input: chunks 0-1 (+ weights) land ~600ns before 2-3,
            # so masks/matmuls on the first half overlap the second transfer.
            # Half B goes through the Pool-engine SWDGE path so its
            # descriptor generation runs in parallel with half A's HWDGE
            # ring instead of serializing behind it.
            nc.sync.dma_start(inba[:], inpa_d[:])
            nc.gpsimd.dma_start(inbb[:], inpb_d[:])

            labh = [inba[:, :HALFW], inbb[:]]  # chunk pairs (0,1), (2,3)
            # fp8 weights: one shared table [0 x8][w_c x7, 0 x9] per chunk;
            # variant A (w in rows 0:7) reads it at base offset 8, variant B
            # (w in rows 8:15) at base offset 0 — the zero runs double as
            # each other's padding, and both keep the dual-fp8 stride of 16
            wf8 = inba[:, HALFW:].bitcast(FP8)  # [128, 72]

            def wview(base):
                return wf8[:, base : base + 32].rearrange(
                    "p (c j) -> p c j", c=2, j=MCOL
                )

            wA = [wview(8), wview(40)]
            wB = [wview(0), wview(32)]

            # DVE stream per half: sc = bf16(CSCALE * lab) (its two fp8 byte
            # planes are two more independent functions), then the single m1
            # is_equal mask.  Only 4 DVE ops total — the chain ends with
            # m1-h2, and bank1 (which feeds the big copy) closes one op
            # earlier on sc-h2.
            m1t = mask_pool.tile([128, LABW], BF16, tag="m1t")
            sct = mask_pool.tile([128, LABW], BF16, tag="sct")

            def half_slice(t, half):
                return t[:, HALFW * half : HALFW * (half + 1)]

            def sc_op(half):
                nc.vector.tensor_scalar(
                    half_slice(sct, half),
                    labh[half],
                    float(CSCALE),
                    None,
                    mybir.AluOpType.mult,
                )

            def m1_op(half):
                nc.vector.tensor_scalar(
                    half_slice(m1t, half),
                    labh[half],
                    1.0,
                    None,
                    mybir.AluOpType.is_equal,
                )

            # half-1's m1 before half-1's sc: bank2 (m1) closes mid-stream
            # and its small copy + Pool DMA tail overlaps bank1's
            sc_op(0)
            m1_op(0)
            m1_op(1)
            sc_op(1)

            DR = mybir.MatmulPerfMode.DoubleRow

            def plane(src_ap, s):
                v = src_ap.bitcast(FP8).rearrange(
                    "p (c r s) -> p c r s", c=2, r=H, s=2
                )
                return v[:, :, :, s]

            # Matmul schedule: (region, weight variant, moving, half, start,
            # stop).  Bank1 = [lab-lo + sc-lo | lab-hi + sc-hi] closes on
            # sc-hi-23; bank2 = m1 closes last but only feeds the small
            # copy + Pool-DMA tail that overlaps out1's HWDGE leg.
            sched = [
                (bank1[0:MCOL, 0:256], wA, plane(labh[0], 0), 0, True, False),
                (bank1[0:MCOL, 256:512], wA, plane(labh[0], 1), 0, False, False),
                (bank1[0:MCOL, 0:256], wB, plane(half_slice(sct, 0), 0), 0, False, False),
                (bank1[0:MCOL, 256:512], wB, plane(half_slice(sct, 0), 1), 0, False, False),
                (bank2[0:MCOL, 0:256], wA, plane(half_slice(m1t, 0), 1), 0, True, False),
                (bank1[0:MCOL, 0:256], wA, plane(labh[1], 0), 1, False, False),
                (bank2[0:MCOL, 0:256], wA, plane(half_slice(m1t, 1), 1), 1, False, True),
                (bank1[0:MCOL, 256:512], wA, plane(labh[1], 1), 1, False, False),
                (bank1[0:MCOL, 0:256], wB, plane(half_slice(sct, 1), 0), 1, False, False),
                (bank1[0:MCOL, 256:512], wB, plane(half_slice(sct, 1), 1), 1, False, True),
            ]
            for out_ap, w, moving, half, start, stop in sched:
                nc.tensor.matmul(
                    out_ap,
                    w[half],
                    moving,
                    start=start,
                    stop=stop,
                    perf_mode=DR,
                    tile_position=(0, 0),
                    skip_group_check=True,
                )

            # PSUM -> SBUF staging on separate tiles (no WAW between them).
            # m4 finishes early: stage it on Act and ship via the Pool SWDGE
            # so the critical bank1 copy (DVE) + its HWDGE DMA never wait.
            nc.scalar.copy(outbb[0:NW8, :], bank2[0:NW8, 0:256])
            nc.vector.tensor_copy(outba[0:MCOL, :], bank1[0:MCOL, :])

            nc.gpsimd.dma_start(out2_d[:], outbb[0:NW8, :])
            nc.sync.dma_start(out1_d[:], outba[0:MCOL, :])

    # Drop the framework's const-tile memsets from the entry block: nothing
    # in this program reads the const APs (all scalar operands are literal
    # immediates), and the 4 serial Pool memsets (95ns each) gate the entry
    # barrier and hence the input DMA issue.
    blk0 = nc.m.functions[0].blocks[0]
    dead = [
        i
        for i, inst in enumerate(blk0.instructions)
        if inst.opcode == "Memset"
        and any("const-" in str(o) for o in inst.outs)
    ]
    if dead:
        used = set()
        for fn in nc.m.functions:
            for blk in fn.blocks:
                for inst in blk.instructions:
                    if inst.opcode == "Memset":
                        continue
                    for ap in list(inst.ins) + list(inst.outs):
                        used.add(str(ap))
        assert not any("const-" in u for u in used), "const APs are used"
        for i in reversed(dead):
            del blk0.instructions[i]
    # With the memsets gone the entry all-engine barrier synchronizes idle
    # engines only; drop it too so the input DMA issues immediately.
    bar = [
        i
        for i, inst in enumerate(blk0.instructions)
        if inst.opcode in ("Drain", "EventSemaphore")
    ]
    for i in reversed(bar):
        del blk0.instructions[i]
    # Exit block: keep the SP queue-drain waits (output-DMA completion) but
    # drop the two all-engine barrier ping-pong rounds and the semaphore
    # range clear — they only matter for re-executing a still-loaded NEFF.
    blk2 = nc.m.functions[0].blocks[2]
    tail = [
        i
        for i, inst in enumerate(blk2.instructions)
        if "barrier" in inst.concise()
        or "EVENT_SEMAPHORE_RANGE_CLEAR" in inst.concise()
        or (inst.opcode == "Drain" and "is_reset_sema=True" in inst.concise())
    ]
    for i in reversed(tail):
        del blk2.instructions[i]
    sp_waits = [
        i
        for i, inst in enumerate(blk2.instructions)
        if inst.opcode == "EventSemaphore" and "DMASW" in inst.concise()
    ]
    for k, i in enumerate(sp_waits):
        if i != k:
            inst = blk2.instructions.pop(i)
            blk2.instructions.insert(k, inst)
    nc.compile()
    return nc


def _fp8_hi_lo(vals: np.ndarray, clip: float = 240.0):
    """Exact v = hi + lo split with both parts fp8 e4m3 (max normal 240)."""
    e4 = ml_dtypes.float8_e4m3
    hi = np.clip(vals, -clip, clip).astype(e4)
    hi64 = hi.astype(np.float64)
    lo = (vals - hi64).astype(e4)
    assert np.all(lo.astype(np.float64) + hi64 == vals), "fp8 split not exact"
    return hi, lo


def _fp8_weights() -> np.ndarray:
    """[128, WCOLS] fp8 shared table: 8 zero cols then per chunk
    [w_c x7, 0 x9] with w = [1,xh,xl,ah,al,bh,bl]; the A/B stationary
    variants are offset views (base 8 / base 0) of this one table."""
    e4 = ml_dtypes.float8_e4m3
    x = np.arange(W, dtype=np.float64)
    xc = x - XC
    xc2 = xc * xc
    a = np.floor(xc2 / 256.0)
    b = xc2 - 256.0 * a
    xh, xl = _fp8_hi_lo(xc)
    ah, al = _fp8_hi_lo(a)
    bh, bl = _fp8_hi_lo(b)
    wreal = np.zeros((W, NW8), dtype=e4)
    wreal[:, 0] = 1.0
    wreal[:, 1] = xh
    wreal[:, 2] = xl
    wreal[:, 3] = ah
    wreal[:, 4] = al
    wreal[:, 5] = bh
    wreal[:, 6] = bl
    wr = wreal.reshape(N_CHUNKS, 128, NW8)
    t = np.zeros((128, WCOLS), dtype=e4)
    for c in range(N_CHUNKS):
        t[:, 8 + MCOL * c : 8 + MCOL * c + NW8] = wr[c]
    return np.ascontiguousarray(t)


def _host_prep(instance_label: np.ndarray):
    lab = np.asarray(instance_label)
    wf8 = _fp8_weights()
    in_maps = []
    for b in range(B):
        lt = lab[b].T.astype(ml_dtypes.bfloat16)  # [W, H]
        lt = lt.reshape(N_CHUNKS, 128, H).transpose(1, 0, 2).reshape(128, LABW)
        inpa = np.empty((128, INWA), dtype=ml_dtypes.bfloat16)
        inpa[:, :HALFW] = lt[:, :HALFW]
        inpa[:, HALFW:].view(np.uint8)[:] = wf8.view(np.uint8)
        in_maps.append({"inpa": inpa, "inpb": np.ascontiguousarray(lt[:, HALFW:])})
    return in_maps


def _decode_moments(raw1: np.ndarray, raw2: np.ndarray) -> np.ndarray:
    """Device outputs -> per-lane moments [3, N_LANES, H] f64.

    raw1 [16, 512]: rows 0:7 = [lab-lo | lab-hi], rows 8:15 =
    [sc-lo | sc-hi]; raw2 [7, 256]: m1 (payload-scaled).
    Moment index: 0 = count, 1 = sum xc, 2 = sum xc^2.
    """
    g1 = raw1.astype(np.float64)
    g2 = raw2.astype(np.float64)

    def comb(t):  # [7, H] fp8-moment rows -> [3, H]
        return np.stack(
            [t[0], t[1] + t[2], 256.0 * (t[3] + t[4]) + t[5] + t[6]]
        )

    t = np.stack(
        [
            comb(g1[0:NW8, 0:256]),  # lab lo plane
            comb(g1[0:NW8, 256:512]),  # lab hi plane
            comb(g1[ROWB : ROWB + NW8, 0:256]),  # sc lo plane
            comb(g1[ROWB : ROWB + NW8, 256:512]),  # sc hi plane
            comb(g2),  # m1 (payload-scaled)
        ]
    )  # [5 funcs, 3 moments, H]
    u = np.linalg.solve(_unmix_matrix(), t.reshape(5, -1)).reshape(t.shape)
    return u.transpose(1, 0, 2)  # [3, L, H]


def _finalize(hnet_params: np.ndarray, moments: np.ndarray) -> np.float32:
    """moments: [B, 3, L, H] f64 (count, S1 about XC, S2 about XC)."""
    p = np.asarray(hnet_params, dtype=np.float64)
    c = moments[:, 0]  # [B, L, H]
    S1c = moments[:, 1]
    S2c = moments[:, 2]
    S1 = S1c + XC * c
    S2 = S2c + 2.0 * XC * S1c + XC * XC * c

    r = np.arange(H, dtype=np.float64)
    p32 = np.asarray(hnet_params, dtype=np.float32)
    den32 = (p32[:, 5:6] * r.astype(np.float32)[None, :]) + np.float32(1.0)
    den = np.where(np.abs(den32) < EPS_DEN, np.float32(EPS_DEN), den32).astype(
        np.float64
    )
    alpha = p[:, 0:1] / den  # [B,H]
    beta = (p[:, 1:2] * r[None, :] + p[:, 2:3]) / den
    yp = (p[:, 3:4] * r[None, :] + p[:, 4:5]) / den

    al = alpha[:, None, :]
    be = beta[:, None, :]
    Sx = al * S1 + be * c
    Sxx = al * al * S2 + 2 * al * be * S1 + be * be * c

    ypb = yp[:, None, :]
    cnt = c.sum(-1)  # [B,L]
    s = np.stack([(c * ypb**k).sum(-1) for k in range(7)], axis=-1)
    t = np.stack([(Sx * ypb**q).sum(-1) for q in range(4)], axis=-1)
    v = (c * np.abs(den)[:, None, :]).sum(-1)

    k = ORDER + 1
    A0 = np.empty((B, N_LANES, k, k))
    for i in range(k):
        for j in range(k):
            A0[:, :, i, j] = s[:, :, 6 - i - j]
    rhs = np.stack([t[:, :, 3 - i] for i in range(k)], axis=-1)
    A = A0 + RIDGE * np.eye(k)
    w = np.linalg.solve(A, rhs[..., None])[..., 0]

    xpred = sum(w[:, :, i, None] * ypb ** (3 - i) for i in range(k))
    rss = (Sxx - 2 * xpred * Sx + xpred * xpred * c).sum(-1)

    cnt_safe = np.maximum(cnt, 1.0)
    lane_loss = (rss / cnt_safe) * (v / cnt_safe)
    valid = (cnt >= ORDER + 1).astype(np.float64)
    nv = valid.sum()
    loss = (valid * lane_loss).sum() / max(nv, 1.0) if nv > 0 else 0.0
    return np.float32(loss)


def _run_device(in_maps, trace: bool = False, trace_cores=None):
    from concourse import bass_utils

    nc = _build_program()
    res = bass_utils.run_bass_kernel_spmd(
        nc,
        in_maps,
        core_ids=list(range(N_CORES)),
        trace=trace,
        trace_cores=trace_cores,
    )
    return res


def kernel(hnet_params: np.ndarray, instance_label: np.ndarray) -> np.ndarray:
    in_maps = _host_prep(instance_label)
    res = _run_device(in_maps)
    moments = np.stack(
        [
            _decode_moments(
                np.asarray(res.results[b]["out1"]),
                np.asarray(res.results[b]["out2"]),
            )
            for b in range(B)
        ]
    )
    return _finalize(hnet_params, moments)


def _golden_moments(lab_b: np.ndarray) -> np.ndarray:
    """Numpy golden for one batch: [3, L, H] exact moments."""
    x = np.arange(W, dtype=np.float64)
    xc = x - XC
    out = np.zeros((3, N_LANES, H))
    for lane in range(N_LANES):
        msk = lab_b == (lane + 1)  # [H, W]
        out[0, lane] = msk.sum(1)
        out[1, lane] = (msk * xc).sum(1)
        out[2, lane] = (msk * xc * xc).sum(1)
    return out


if __name__ == "__main__":
    from concourse.bass_interp import CoreSim

    rng = np.random.default_rng(0)
    lab_full = rng.integers(0, 6, size=(B, H, W)).astype(np.int64)
    in_maps = _host_prep(lab_full)

    nc = _build_program()
    sim = CoreSim(nc)
    sim.tensor("inpa")[:] = in_maps[0]["inpa"]
    sim.tensor("inpb")[:] = in_maps[0]["inpb"]
    sim.simulate()
    mom = _decode_moments(
        np.asarray(sim.tensor("out1")), np.asarray(sim.tensor("out2"))
    )

    golden = _golden_moments(lab_full[0])
    err = np.abs(mom - golden)
    rel = err.max() / max(np.abs(golden).max(), 1)
    print("max abs err:", err.max(), "max rel:", rel)
    assert rel < 1e-6, "CoreSim moments mismatch"
    print("CoreSim moments check PASSED")
